# revision 22
# baseline (speedup 1.0000x reference)
"""Causal self-attention (B=2, T=2048, C=1024, H=16, D=64) on 8 TRN2 cores.

Sharding: batch across 2 groups of 4 cores; 4 heads per core within a group
(Megatron column-parallel QKV). After attention, AllGather the per-head
outputs within each group (fp8 value+residual pair), then column-parallel
c_proj (each core computes 256 output columns for all T).

fp8 strategy (all scale factors are exact powers of two, folded into
existing copies / the exp scale, so descale costs nothing):
  X' = x^T*AX quantized to fp8e4m3 host-side as x8 + residual dx8.
  W' = w*AW quantized host-side (wqk8 single; wv8+dwv8, wp8+dwp8 pairs).
  QK  = (x8 + dx8)@wqk8          2-pass DoubleRow (w-quant error is washed
                                 out by softmax; x-residual is not, so kept)
  V   = x8@wv8 + x8@dwv8 + dx8@wv8   3-pass DoubleRow (~exact: only the
                                 2nd-order dx8@dwv8 term is dropped)
  S   = q8^T k8 plain fp8 over D=64 (q,k requantized fp8*AS; softmax
                                 washes the quant error; DoubleRow scores
                                 measured slower on HW due to repack DMAs)
  att@V in fp32r (fp8 attention weights would cost ~1.4e-2 error alone).
  y -> y8 + dy8 fp8 pair (~exact); two packed [1024,512] AllGathers (chunk
  pairs) -- fewer group-sync points measured faster than four.
  proj = y8@wp8 + y8@dwp8 + dy8@wp8  3-pass DoubleRow (~exact)

Measured end-to-end rel err ~1.3e-2 vs the 2e-2 gate (numpy model 1.37e-2).

Per-core PE work drops from ~278K cycle-rows (all fp32r) to ~200K:
QKV 32.8K + V 24.6K + S 34.8K + att@V 69.6K(+diag) + norm 8.2K + proj 24.6K.

QKV (stage A) and attention (stage B) are emitted interleaved per t-chunk;
each chunk's normalize + AllGather is deferred by one chunk and issued
mid-compute so the first three AllGathers hide under attention. The
score/accumulator PSUM pools close right after attention so proj quarters
0-2 run before the final normalize+AllGather.

Output per core: out_c [256, 2048] = out^T columns slice; host reassembles.
"""

import sys

sys.path.insert(0, "/opt/trn_rl_repo")

from contextlib import ExitStack

import numpy as np
import ml_dtypes

B, T, C, H, D = 2, 2048, 1024, 16, 64
NCORES = 8
HL = 4   # heads per core
NKC = 8  # contraction chunks (C / 128)
NCH = 4  # t chunks (T / 512)
NST = 16  # s tiles (T / 128)

AX = 8.0    # x fp8 scale
AW = 256.0  # weight fp8 scale
AS = 16.0   # q/k score-input fp8 scale
AY = 32.0   # y fp8 scale

E4NP = ml_dtypes.float8_e4m3

_prog_cache = {}

# ablation flags for perf experiments (not used in production path)
VARIANT = {"no_ag": False, "no_proj": False, "scores_nodr": True,
           "ag_sync": False, "bufs4": False, "big_nodr": False,
           "ag2": True}


def build_program(reps=1, qk_bias=False, out_bias=False):
    key = (reps, qk_bias, out_bias, tuple(sorted(VARIANT.items())))
    if key in _prog_cache:
        return _prog_cache[key]

    from concourse import bacc, mybir
    import concourse.tile as tile

    F32 = mybir.dt.float32
    F32R = mybir.dt.float32r
    F8 = mybir.dt.float8e4

    nc = bacc.Bacc(num_devices=NCORES)

    xt8 = nc.declare_dram_parameter("xt8", [128, NKC, T], F8, isOutput=False)
    dxt8 = nc.declare_dram_parameter("dxt8", [128, NKC, T], F8, isOutput=False)
    wqk8 = nc.declare_dram_parameter("wqk8", [128, NKC, 512], F8, isOutput=False)
    wv8 = nc.declare_dram_parameter("wv8", [128, NKC, 256], F8, isOutput=False)
    dwv8 = nc.declare_dram_parameter("dwv8", [128, NKC, 256], F8, isOutput=False)
    wp8 = nc.declare_dram_parameter("wp8", [128, NKC, 256], F8, isOutput=False)
    dwp8 = nc.declare_dram_parameter("dwp8", [128, NKC, 256], F8, isOutput=False)
    tri = nc.declare_dram_parameter("tri", [128, 128], F32R, isOutput=False)
    vone = nc.declare_dram_parameter("vone", [128, NST, 4, 1], F32R, isOutput=False)
    sel = nc.declare_dram_parameter("sel", [16, 16, 64], F32R, isOutput=False)
    if qk_bias:
        bqk = nc.declare_dram_parameter("bqk", [128, 4], F32, isOutput=False)
    if out_bias:
        bout = nc.declare_dram_parameter("bout", [128, 2], F32, isOutput=False)
    out_c = nc.declare_dram_parameter("out_c", [256, T], F32, isOutput=True)

    with tile.TileContext(nc) as tc:
        with ExitStack() as outer:
            const = outer.enter_context(tc.tile_pool(name="const", bufs=1))
            wqk8_sb = const.tile([128, NKC, 512], F8)
            wv8_sb = const.tile([128, NKC, 256], F8)
            dwv8_sb = const.tile([128, NKC, 256], F8)
            wp8_sb = const.tile([128, NKC, 256], F8)
            dwp8_sb = const.tile([128, NKC, 256], F8)
            tri_sb = const.tile([128, 128], F32R)
            sel_sb = const.tile([16, 16, 64], F32R)
            nc.scalar.dma_start(wqk8_sb[:], wqk8[:])
            nc.scalar.dma_start(wv8_sb[:], wv8[:])
            nc.scalar.dma_start(dwv8_sb[:], dwv8[:])
            nc.scalar.dma_start(wp8_sb[:], wp8[:])
            nc.scalar.dma_start(dwp8_sb[:], dwp8[:])
            nc.scalar.dma_start(tri_sb[:], tri[:])
            nc.scalar.dma_start(sel_sb[:], sel[:])
            bqk_sb = bout_sb = None
            if qk_bias:
                bqk_sb = const.tile([128, 4], F32)
                nc.scalar.dma_start(bqk_sb[:], bqk[:])
            if out_bias:
                bout_sb = const.tile([128, 2], F32)
                nc.scalar.dma_start(bout_sb[:], bout[:])

            for rep in range(reps):
                _emit_body(
                    nc, tc, mybir, rep,
                    xt8=xt8, dxt8=dxt8, vone=vone, out_c=out_c,
                    wqk8_sb=wqk8_sb, wv8_sb=wv8_sb, dwv8_sb=dwv8_sb,
                    wp8_sb=wp8_sb, dwp8_sb=dwp8_sb,
                    tri_sb=tri_sb, sel_sb=sel_sb,
                    bqk_sb=bqk_sb, bout_sb=bout_sb,
                )

    nc.finalize()
    _prog_cache[key] = nc
    return nc


def _emit_body(nc, tc, mybir, rep, *, xt8, dxt8, vone, out_c, wqk8_sb,
               wv8_sb, dwv8_sb, wp8_sb, dwp8_sb, tri_sb, sel_sb,
               bqk_sb, bout_sb):
    F32 = mybir.dt.float32
    F32R = mybir.dt.float32r
    F8 = mybir.dt.float8e4
    AF = mybir.ActivationFunctionType
    MUL = mybir.AluOpType.mult
    SUB = mybir.AluOpType.subtract
    ADD = mybir.AluOpType.add
    DR = mybir.MatmulPerfMode.DoubleRow
    R = f"r{rep}"

    QK_SCALE = AS / (AX * AW)     # PSUM qk -> fp8 tile scale
    V_SCALE = 1.0 / (AX * AW)     # PSUM v -> f32r v_sb
    EXP_SCALE = 0.125 / (AS * AS)  # scores PSUM -> exp
    OUT_SCALE = 1.0 / (AY * AW)   # proj PSUM -> out

    with ExitStack() as persist:
        stP = persist.enter_context(tc.tile_pool(name=f"stP{R}", bufs=1))
        dpool = persist.enter_context(
            tc.tile_pool(name=f"dram{R}", bufs=1, space="DRAM"))
        if VARIANT["scores_nodr"]:
            # plain fp8 scores (K=64): [128=(hh,d), m, t] like v1's qk_sb
            q2_sb = stP.tile([128, 2, T], F8, name=f"q2_sb{R}")
            k2_sb = stP.tile([128, 2, T], F8, name=f"k2_sb{R}")
        else:
            # packed fp8 q/k for DoubleRow scores: [32, head, slot, t],
            # contraction d = 2p + slot
            q2_sb = stP.tile([32, 4, 2, T], F8, name=f"q2_sb{R}")
            k2_sb = stP.tile([32, 4, 2, T], F8, name=f"k2_sb{R}")
        # V natural f32r, 65-stride per head (65th col = ones)
        v_sb = stP.tile([128, NST, 260], F32R, name=f"v_sb{R}")
        # y raw + denominator row (partition 64), per t-chunk; blocks = head
        yraw_q = [
            stP.tile([65, 4, 512], F32R, name=f"yraw{R}_{q}")
            for q in range(NCH)
        ]
        # fp8 y8 (rows 0-255) + dy8 (rows 256-511) packed per chunk
        if VARIANT["ag2"]:
            y_in_q = [
                dpool.tile([1024, 512], F8, name=f"y_in{R}_{q}")
                for q in range(2)
            ]
            y_full_q = [
                dpool.tile([4096, 512], F8, name=f"y_full{R}_{q}")
                for q in range(2)
            ]
        else:
            y_in_q = [
                dpool.tile([512, 512], F8, name=f"y_in{R}_{q}")
                for q in range(NCH)
            ]
            y_full_q = [
                dpool.tile([2048, 512], F8, name=f"y_full{R}_{q}")
                for q in range(NCH)
            ]

        with (
            tc.tile_pool(name=f"stAB{R}", bufs=1) as stAB,
            tc.tile_pool(name=f"psA{R}", bufs=1, space="PSUM") as psA,
        ):
            sy_ctx = ExitStack()
            psS = sy_ctx.enter_context(
                tc.tile_pool(name=f"psS{R}", bufs=1, space="PSUM"))
            psY = sy_ctx.enter_context(
                tc.tile_pool(name=f"psY{R}", bufs=1, space="PSUM"))
            vview = v_sb[:].rearrange("p t (h x) -> p t h x", h=4)
            nc.scalar.dma_start(vview[:, :, :, 64:65], vone[:])
            xt_t, dxt_t = [], []
            for n in range(NCH):
                xtile = stAB.tile([128, NKC, 512], F8, tag="xt", bufs=3,
                                  name=f"xt_t{R}_{n}")
                nc.sync.dma_start(xtile[:], xt8[:, :, n * 512:(n + 1) * 512])
                xt_t.append(xtile)
                dxtile = stAB.tile([128, NKC, 512], F8, tag="dxt", bufs=3,
                                   name=f"dxt_t{R}_{n}")
                nc.sync.dma_start(dxtile[:], dxt8[:, :, n * 512:(n + 1) * 512])
                dxt_t.append(dxtile)

            def emit_norm_ag(n, yraw, r4):
                tmp = stAB.tile([64, 4, 512], F32, tag="tmp", bufs=2,
                                name=f"tmp{R}_{n}")
                for h in range(4):
                    rb = psA.tile([64, 512], F32, tag="pA", bufs=2,
                                  name=f"rb{R}_{n}_{h}")
                    nc.tensor.matmul(
                        rb[:], sel_sb[0:4, h, :], r4[:],
                        start=True, stop=True,
                    )
                    nc.vector.tensor_tensor(
                        tmp[:, h, :], yraw[0:64, h, :], rb[:], MUL,
                    )
                y8t = stAB.tile([64, 4, 512], F8, tag="y8", bufs=2,
                                name=f"y8{R}_{n}")
                nc.vector.tensor_copy(y8t[:], tmp[:])
                dy8t = stAB.tile([64, 4, 512], F8, tag="dy8", bufs=2,
                                 name=f"dy8{R}_{n}")
                nc.vector.tensor_tensor(dy8t[:], tmp[:], y8t[:], SUB)
                if VARIANT["ag2"]:
                    tin = y_in_q[n // 2]
                    b0 = 512 * (n % 2)
                else:
                    tin = y_in_q[n]
                    b0 = 0
                nc.scalar.dma_start(
                    tin[b0:b0 + 256, :].rearrange("(h p) u -> p h u", p=64),
                    y8t[:])
                nc.scalar.dma_start(
                    tin[b0 + 256:b0 + 512, :].rearrange(
                        "(h p) u -> p h u", p=64),
                    dy8t[:])
                if not VARIANT["no_ag"] and (
                        not VARIANT["ag2"] or n % 2 == 1):
                    nc.gpsimd.collective_compute(
                        "AllGather",
                        mybir.AluOpType.bypass,
                        replica_groups=[[0, 1, 2, 3], [4, 5, 6, 7]],
                        ins=[tin[:]],
                        outs=[y_full_q[n // 2 if VARIANT["ag2"] else n][:]],
                    )

            pending = None
            for n in range(NCH):
                    yraw = yraw_q[n]
                    # --- QKV q/k part: 2-pass fp8 DoubleRow ---
                    for m in range(4):
                        ps = psA.tile([128, 512], F32, tag="pA", bufs=2,
                                      name=f"qkvps{R}_{n}_{m}")
                        for xt_pass, first, last in (
                            (xt_t[n], True, False),
                            (dxt_t[n], False, True),
                        ):
                            if VARIANT["big_nodr"]:
                                for kc in range(8):
                                    nc.tensor.matmul(
                                        ps[:],
                                        wqk8_sb[:, kc,
                                                m * 128:(m + 1) * 128],
                                        xt_pass[:, kc, :],
                                        start=(first and kc == 0),
                                        stop=(last and kc == 7),
                                    )
                            else:
                                for kcp in range(4):
                                    nc.tensor.matmul(
                                        ps[:],
                                        wqk8_sb[:, 2 * kcp:2 * kcp + 2,
                                                m * 128:(m + 1) * 128],
                                        xt_pass[:, 2 * kcp:2 * kcp + 2, :],
                                        start=(first and kcp == 0),
                                        stop=(last and kcp == 3),
                                        perf_mode=DR,
                                    )
                        if VARIANT["scores_nodr"]:
                            dst = q2_sb if m < 2 else k2_sb
                            tgt = dst[:, m % 2, n * 512:(n + 1) * 512]
                            if bqk_sb is not None:
                                nc.vector.tensor_scalar(
                                    tgt, ps[:], QK_SCALE,
                                    bqk_sb[:, m:m + 1], MUL, ADD,
                                )
                            else:
                                nc.vector.tensor_scalar_mul(
                                    tgt, ps[:], QK_SCALE)
                            continue
                        qk8t = stAB.tile(
                            [128, 512], F8, tag="qk8",
                            bufs=4 if VARIANT["bufs4"] else 3,
                            name=f"qk8{R}_{n}_{m}")
                        if bqk_sb is not None:
                            nc.vector.tensor_scalar(
                                qk8t[:], ps[:], QK_SCALE,
                                bqk_sb[:, m:m + 1], MUL, ADD,
                            )
                        else:
                            nc.vector.tensor_scalar_mul(
                                qk8t[:], ps[:], QK_SCALE)
                        # repack head halves -> [32, h, slot, t] with the
                        # d = 2p + slot packing (flat DMA orders match, so
                        # one dma_start per head)
                        dst = q2_sb if m < 2 else k2_sb
                        for hh in range(2):
                            h = 2 * (m % 2) + hh
                            nc.sync.dma_start(
                                dst[:, h, :, n * 512:(n + 1) * 512],
                                qk8t[64 * hh:64 * hh + 64, :],
                            )
                    # --- V: 3-pass fp8 DoubleRow ---
                    for tt in range(4):
                        st = 4 * n + tt
                        psv = psA.tile([128, 512], F32, tag="pA", bufs=2,
                                       name=f"vps{R}_{st}")
                        passes = (
                            (xt_t[n], wv8_sb, True, False),
                            (xt_t[n], dwv8_sb, False, False),
                            (dxt_t[n], wv8_sb, False, True),
                        )
                        for xa, wa, first, last in passes:
                            if VARIANT["big_nodr"]:
                                for kc in range(8):
                                    nc.tensor.matmul(
                                        psv[:, 0:256],
                                        xa[:, kc, tt * 128:(tt + 1) * 128],
                                        wa[:, kc, :],
                                        start=(first and kc == 0),
                                        stop=(last and kc == 7),
                                    )
                            else:
                                for kcp in range(4):
                                    nc.tensor.matmul(
                                        psv[:, 0:256],
                                        xa[:, 2 * kcp:2 * kcp + 2,
                                           tt * 128:(tt + 1) * 128],
                                        wa[:, 2 * kcp:2 * kcp + 2, :],
                                        start=(first and kcp == 0),
                                        stop=(last and kcp == 3),
                                        perf_mode=DR,
                                    )
                        nc.vector.tensor_scalar_mul(
                            v_sb[:, st, :].rearrange(
                                "p (h x) -> p h x", h=4)[:, :, 0:64],
                            psv[:, 0:256].rearrange("p (h x) -> p h x", h=4),
                            V_SCALE,
                        )

                    if pending is not None:
                        emit_norm_ag(*pending)
                        pending = None

                    n_st = 4 * n + 4
                    for p in range(2):
                        ype = psY.tile([65, 512], F32, tag="ye", bufs=1,
                                       name=f"ype{R}_{n}_{p}")
                        ypo = psY.tile([65, 512], F32, tag="yo", bufs=1,
                                       name=f"ypo{R}_{n}_{p}")
                        for st in range(n_st):
                            diag = st - 4 * n
                            toff = 128 * diag if diag >= 0 else 0
                            scp = psS.tile([128, 1024], F32, tag="sc", bufs=2,
                                           name=f"scp{R}_{n}_{p}_{st}")
                            es = stAB.tile(
                                [128, 1024], F32R, tag="es",
                                bufs=4 if VARIANT["bufs4"] else 3,
                                name=f"es{R}_{n}_{p}_{st}")
                            for hp in range(2):
                                h = 2 * p + hp
                                if VARIANT["scores_nodr"]:
                                    nc.tensor.matmul(
                                        scp[:, hp * 512 + toff:
                                            (hp + 1) * 512],
                                        k2_sb[64 * hp:64 * hp + 64, p,
                                              st * 128:(st + 1) * 128],
                                        q2_sb[64 * hp:64 * hp + 64, p,
                                              n * 512 + toff:(n + 1) * 512],
                                        start=True, stop=True,
                                    )
                                else:
                                    nc.tensor.matmul(
                                        scp[:, hp * 512 + toff:
                                            (hp + 1) * 512],
                                        k2_sb[:, h, :,
                                              st * 128:(st + 1) * 128],
                                        q2_sb[:, h, :,
                                              n * 512 + toff:(n + 1) * 512],
                                        start=True, stop=True,
                                        perf_mode=DR,
                                    )
                            if diag < 0:
                                nc.scalar.activation(
                                    es[:], scp[:], AF.Exp, scale=EXP_SCALE
                                )
                            else:
                                esv = es[:].rearrange(
                                    "p (hp u) -> p hp u", hp=2)
                                scv = scp[:].rearrange(
                                    "p (hp u) -> p hp u", hp=2)
                                nc.scalar.activation(
                                    esv[:, :, toff:512], scv[:, :, toff:512],
                                    AF.Exp, scale=EXP_SCALE,
                                )
                                for hp in range(2):
                                    nc.vector.tensor_tensor(
                                        es[:, hp * 512 + toff:
                                           hp * 512 + toff + 128],
                                        es[:, hp * 512 + toff:
                                           hp * 512 + toff + 128],
                                        tri_sb[:], MUL,
                                    )
                            for hp, yp in ((0, ype), (1, ypo)):
                                h = 2 * p + hp
                                nc.tensor.matmul(
                                    yp[:, toff:512],
                                    v_sb[:, st, 65 * h:65 * h + 65],
                                    es[:, hp * 512 + toff:(hp + 1) * 512],
                                    start=(st == 0), stop=(st == n_st - 1),
                                )
                        for hp, yp in ((0, ype), (1, ypo)):
                            h = 2 * p + hp
                            nc.vector.tensor_copy(yraw[:, h, :], yp[:])

                    den4 = stAB.tile([4, 512], F32R, tag="den4", bufs=2,
                                     name=f"den4{R}_{n}")
                    nc.scalar.dma_start(den4[:], yraw[64:65, :, :])
                    rf = stAB.tile([4, 512], F32, tag="rf", bufs=2,
                                   name=f"rf{R}_{n}")
                    nc.vector.reciprocal_approx_fast(
                        rf[:], den4[:].bitcast(F32))
                    r4 = stAB.tile([4, 512], F32R, tag="r4", bufs=2,
                                   name=f"r4{R}_{n}")
                    nc.vector.tensor_scalar_mul(r4[:], rf[:], AY)
                    pending = (n, yraw, r4)

            # free the attention score/accumulator banks (6), keep psA
            # open for the final normalize; proj 0-2 draws only from the
            # freed space, so it is NOT gated on the last recip chain.
            sy_ctx.close()

            def emit_proj(q, psP):
                pp0 = psP.tile([128, 512], F32, tag="pp0", bufs=2,
                               name=f"pp0{R}_{q}")
                pp1 = psP.tile([128, 512], F32, tag="pp1", bufs=2,
                               name=f"pp1{R}_{q}")
                for kcp in range(4):
                    # [p, slot j, w (y8 vs dy8), t]; one DMA per rank block
                    ydf = stAB.tile([128, 2, 2, 512], F8, tag="yf", bufs=4,
                                    name=f"ydf{R}_{q}_{kcp}")
                    if q < 3:
                        dma_eng = nc.sync
                    else:
                        dma_eng = nc.sync if kcp % 2 == 0 else nc.scalar
                    for w in range(2):
                        if VARIANT["ag2"]:
                            ysrc = y_full_q[q // 2]
                            base = 1024 * kcp + 512 * (q % 2) + 256 * w
                        else:
                            ysrc = y_full_q[q]
                            base = 512 * kcp + 256 * w
                        dma_eng.dma_start(
                            ydf[:, :, w, :],
                            ysrc[base:base + 256, :].rearrange(
                                "(j p) u -> p j u", j=2),
                        )
                    yf = ydf[:, :, 0, :]
                    dyf = ydf[:, :, 1, :]
                    for m2, pp in ((0, pp0), (1, pp1)):
                        wsl = slice(m2 * 128, (m2 + 1) * 128)
                        if VARIANT["big_nodr"]:
                            for j in range(2):
                                kc = 2 * kcp + j
                                nc.tensor.matmul(
                                    pp[:], wp8_sb[:, kc, wsl],
                                    ydf[:, j, 0, :],
                                    start=(kcp == 0 and j == 0), stop=False,
                                )
                                nc.tensor.matmul(
                                    pp[:], dwp8_sb[:, kc, wsl],
                                    ydf[:, j, 0, :],
                                    start=False, stop=False,
                                )
                                nc.tensor.matmul(
                                    pp[:], wp8_sb[:, kc, wsl],
                                    ydf[:, j, 1, :],
                                    start=False,
                                    stop=(kcp == 3 and j == 1),
                                )
                        else:
                            ksl = slice(2 * kcp, 2 * kcp + 2)
                            nc.tensor.matmul(
                                pp[:], wp8_sb[:, ksl, wsl], yf,
                                start=(kcp == 0), stop=False, perf_mode=DR,
                            )
                            nc.tensor.matmul(
                                pp[:], dwp8_sb[:, ksl, wsl], yf,
                                start=False, stop=False, perf_mode=DR,
                            )
                            nc.tensor.matmul(
                                pp[:], wp8_sb[:, ksl, wsl], dyf,
                                start=False, stop=(kcp == 3), perf_mode=DR,
                            )
                out_sb = stAB.tile([128, 2, 512], F32, tag="out_sb", bufs=2,
                                   name=f"out_sb{R}_{q}")
                for m2, pp in ((0, pp0), (1, pp1)):
                    if bout_sb is not None:
                        nc.vector.tensor_scalar(
                            out_sb[:, m2, :], pp[:], OUT_SCALE,
                            bout_sb[:, m2:m2 + 1], MUL, ADD,
                        )
                    else:
                        nc.vector.tensor_scalar_mul(
                            out_sb[:, m2, :], pp[:], OUT_SCALE)
                nc.sync.dma_start(
                    out_c[:, q * 512:(q + 1) * 512].rearrange(
                        "(m p) t -> p m t", p=128),
                    out_sb[:],
                )

            with tc.tile_pool(name=f"psP{R}", bufs=1, space="PSUM") as psP:
                if VARIANT["no_proj"]:
                    emit_norm_ag(*pending)
                    junk = stAB.tile([128, 2, 512], F32, tag="out_sb", bufs=2,
                                     name=f"junk{R}")
                    nc.vector.memset(junk[:], 0.0)
                    for q in range(NCH):
                        nc.sync.dma_start(
                            out_c[:, q * 512:(q + 1) * 512].rearrange(
                                "(m p) t -> p m t", p=128),
                            junk[:],
                        )
                elif VARIANT["ag2"]:
                    emit_proj(0, psP)
                    emit_proj(1, psP)
                    emit_norm_ag(*pending)
                    emit_proj(2, psP)
                    emit_proj(3, psP)
                else:
                    for q in range(3):
                        emit_proj(q, psP)
                    emit_norm_ag(*pending)
                    emit_proj(3, psP)


def _chunked(a):
    """(C, X) -> [128, C/128, X] contraction-chunked layout."""
    c, x = a.shape
    return np.ascontiguousarray(
        a.reshape(c // 128, 128, x).transpose(1, 0, 2)
    )


def _q8(a):
    return np.asarray(a, dtype=E4NP)


def make_in_maps(x, w_attn, b_attn, w_proj, b_proj):
    x = np.asarray(x, dtype=np.float32)
    w_attn = np.asarray(w_attn, dtype=np.float32)
    b_attn = np.asarray(b_attn, dtype=np.float32)
    w_proj = np.asarray(w_proj, dtype=np.float32)
    b_proj = np.asarray(b_proj, dtype=np.float32)

    qk_bias = bool(np.any(b_attn[: 2 * C] != 0))
    b_out_full = b_attn[2 * C:] @ w_proj + b_proj  # V bias folds through
    out_bias = bool(np.any(b_out_full != 0))

    tri_np = np.triu(np.ones((128, 128), np.float32))
    vone_np = np.ones((128, NST, 4, 1), np.float32)
    sel_np = np.zeros((16, 16, 64), np.float32)
    for i in range(16):
        sel_np[i, i, :] = 1.0

    xt8_g, dxt8_g = [], []
    for g in range(B):
        X = np.ascontiguousarray(x[g].T) * AX
        x8 = _q8(X)
        dx8 = _q8(X - x8.astype(np.float32))
        xt8_g.append(_chunked(x8.astype(np.float32)).astype(E4NP))
        dxt8_g.append(_chunked(dx8.astype(np.float32)).astype(E4NP))

    in_maps = []
    for core in range(NCORES):
        g, r = core // 4, core % 4
        h0 = r * HL
        qcols = slice(h0 * D, (h0 + HL) * D)
        kcols = slice(C + h0 * D, C + (h0 + HL) * D)
        vcols = slice(2 * C + h0 * D, 2 * C + (h0 + HL) * D)
        wqk_s = np.concatenate(
            [w_attn[:, qcols], w_attn[:, kcols]], axis=1) * AW
        wqk8_np = _chunked(_q8(wqk_s).astype(np.float32)).astype(E4NP)
        wv_s = np.ascontiguousarray(w_attn[:, vcols]) * AW
        wv8 = _q8(wv_s)
        dwv8 = _q8(wv_s - wv8.astype(np.float32))
        wp_s = np.ascontiguousarray(w_proj[:, 256 * r: 256 * (r + 1)]) * AW
        wp8 = _q8(wp_s)
        dwp8 = _q8(wp_s - wp8.astype(np.float32))
        m = {
            "xt8": xt8_g[g],
            "dxt8": dxt8_g[g],
            "wqk8": wqk8_np,
            "wv8": _chunked(wv8.astype(np.float32)).astype(E4NP),
            "dwv8": _chunked(dwv8.astype(np.float32)).astype(E4NP),
            "wp8": _chunked(wp8.astype(np.float32)).astype(E4NP),
            "dwp8": _chunked(dwp8.astype(np.float32)).astype(E4NP),
            "tri": tri_np,
            "vone": vone_np,
            "sel": sel_np,
        }
        if qk_bias:
            bq = np.concatenate([b_attn[qcols], b_attn[kcols]]) * AS  # (512,)
            m["bqk"] = np.ascontiguousarray(
                bq.reshape(4, 128).T.astype(np.float32))
        if out_bias:
            bo = b_out_full[256 * r: 256 * (r + 1)]
            m["bout"] = np.ascontiguousarray(
                bo.reshape(2, 128).T.astype(np.float32))
        in_maps.append(m)
    return in_maps, qk_bias, out_bias


def assemble_output(results):
    out = np.empty((B, T, C), dtype=np.float32)
    for core in range(NCORES):
        g, r = core // 4, core % 4
        out[g][:, 256 * r: 256 * (r + 1)] = results[core]["out_c"].T
    return out


def kernel(x, w_attn, b_attn, w_proj, b_proj):
    from concourse.bass_utils import run_bass_kernel_spmd

    in_maps, qk_bias, out_bias = make_in_maps(
        x, w_attn, b_attn, w_proj, b_proj)
    nc = build_program(reps=1, qk_bias=qk_bias, out_bias=out_bias)
    res = run_bass_kernel_spmd(nc, in_maps, list(range(NCORES)))
    return assemble_output(res.results)


# revision 25
# speedup vs baseline: 1.0502x; 1.0502x over previous
"""Causal self-attention (B=2, T=2048, C=1024, H=16, D=64) on 8 TRN2 cores.

Sharding: batch across 2 groups of 4 cores; 4 heads per core within a group
(Megatron column-parallel QKV). After attention, AllGather the per-head
outputs within each group (fp8 value+residual pair), then column-parallel
c_proj (each core computes 256 output columns for all T).

fp8 strategy (all scale factors are exact powers of two, folded into
existing copies / the exp scale, so descale costs nothing):
  X' = x^T*AX quantized to fp8e4m3 host-side as x8 + residual dx8.
  W' = w*AW quantized host-side (wqk8 single; wv8+dwv8, wp8+dwp8 pairs).
  QK  = (x8 + dx8)@wqk8          2-pass DoubleRow (w-quant error is washed
                                 out by softmax; x-residual is not, so kept)
  V   = x8@wv8 + x8@dwv8 + dx8@wv8   3-pass DoubleRow (~exact: only the
                                 2nd-order dx8@dwv8 term is dropped)
  S   = q8^T k8 plain fp8 over D=64 (q,k requantized fp8*AS; softmax
                                 washes the quant error; DoubleRow scores
                                 measured slower on HW due to repack DMAs)
  att@V in fp32r (fp8 attention weights would cost ~1.4e-2 error alone).
  y -> y8 + dy8 fp8 pair (~exact); two packed [1024,512] AllGathers (chunk
  pairs) -- fewer group-sync points measured faster than four.
  proj = y8@wp8 + y8@dwp8 + dy8@wp8  3-pass DoubleRow (~exact)

Measured end-to-end rel err ~1.3e-2 vs the 2e-2 gate (numpy model 1.37e-2).

Per-core PE work drops from ~278K cycle-rows (all fp32r) to ~200K:
QKV 32.8K + V 24.6K + S 34.8K + att@V 69.6K(+diag) + norm 8.2K + proj 24.6K.

QKV (stage A) and attention (stage B) are emitted interleaved per t-chunk;
each chunk's normalize + AllGather is deferred by one chunk and issued
mid-compute so the first three AllGathers hide under attention. The
score/accumulator PSUM pools close right after attention so proj quarters
0-2 run before the final normalize+AllGather.

Output per core: out_c [256, 2048] = out^T columns slice; host reassembles.
"""

import sys

sys.path.insert(0, "/opt/trn_rl_repo")

from contextlib import ExitStack

import numpy as np
import ml_dtypes

B, T, C, H, D = 2, 2048, 1024, 16, 64
NCORES = 8
HL = 4   # heads per core
NKC = 8  # contraction chunks (C / 128)
NCH = 4  # t chunks (T / 512)
NST = 16  # s tiles (T / 128)

AX = 8.0    # x fp8 scale
AW = 256.0  # weight fp8 scale
AS = 16.0   # q/k score-input fp8 scale
AY = 32.0   # y fp8 scale

E4NP = ml_dtypes.float8_e4m3

_prog_cache = {}

# ablation flags for perf experiments (not used in production path)
VARIANT = {"no_ag": False, "no_proj": False, "scores_nodr": True,
           "ag_sync": False, "bufs4": False, "big_nodr": False,
           "ag2": True, "swpipe": True}


def build_program(reps=1, qk_bias=False, out_bias=False):
    key = (reps, qk_bias, out_bias, tuple(sorted(VARIANT.items())))
    if key in _prog_cache:
        return _prog_cache[key]

    from concourse import bacc, mybir
    import concourse.tile as tile

    F32 = mybir.dt.float32
    F32R = mybir.dt.float32r
    F8 = mybir.dt.float8e4

    nc = bacc.Bacc(num_devices=NCORES)

    xt8 = nc.declare_dram_parameter("xt8", [128, NKC, T], F8, isOutput=False)
    dxt8 = nc.declare_dram_parameter("dxt8", [128, NKC, T], F8, isOutput=False)
    wqk8 = nc.declare_dram_parameter("wqk8", [128, NKC, 512], F8, isOutput=False)
    wv8 = nc.declare_dram_parameter("wv8", [128, NKC, 256], F8, isOutput=False)
    dwv8 = nc.declare_dram_parameter("dwv8", [128, NKC, 256], F8, isOutput=False)
    wp8 = nc.declare_dram_parameter("wp8", [128, NKC, 256], F8, isOutput=False)
    dwp8 = nc.declare_dram_parameter("dwp8", [128, NKC, 256], F8, isOutput=False)
    tri = nc.declare_dram_parameter("tri", [128, 128], F32R, isOutput=False)
    vone = nc.declare_dram_parameter("vone", [128, NST, 4, 1], F32R, isOutput=False)
    sel = nc.declare_dram_parameter("sel", [16, 16, 64], F32R, isOutput=False)
    if qk_bias:
        bqk = nc.declare_dram_parameter("bqk", [128, 4], F32, isOutput=False)
    if out_bias:
        bout = nc.declare_dram_parameter("bout", [128, 2], F32, isOutput=False)
    out_c = nc.declare_dram_parameter("out_c", [256, T], F32, isOutput=True)

    with tile.TileContext(nc) as tc:
        with ExitStack() as outer:
            const = outer.enter_context(tc.tile_pool(name="const", bufs=1))
            wqk8_sb = const.tile([128, NKC, 512], F8)
            wv8_sb = const.tile([128, NKC, 256], F8)
            dwv8_sb = const.tile([128, NKC, 256], F8)
            wp8_sb = const.tile([128, NKC, 256], F8)
            dwp8_sb = const.tile([128, NKC, 256], F8)
            tri_sb = const.tile([128, 128], F32R)
            sel_sb = const.tile([16, 16, 64], F32R)
            nc.scalar.dma_start(wqk8_sb[:], wqk8[:])
            nc.scalar.dma_start(wv8_sb[:], wv8[:])
            nc.scalar.dma_start(dwv8_sb[:], dwv8[:])
            nc.scalar.dma_start(wp8_sb[:], wp8[:])
            nc.scalar.dma_start(dwp8_sb[:], dwp8[:])
            nc.scalar.dma_start(tri_sb[:], tri[:])
            nc.scalar.dma_start(sel_sb[:], sel[:])
            bqk_sb = bout_sb = None
            if qk_bias:
                bqk_sb = const.tile([128, 4], F32)
                nc.scalar.dma_start(bqk_sb[:], bqk[:])
            if out_bias:
                bout_sb = const.tile([128, 2], F32)
                nc.scalar.dma_start(bout_sb[:], bout[:])

            for rep in range(reps):
                _emit_body(
                    nc, tc, mybir, rep,
                    xt8=xt8, dxt8=dxt8, vone=vone, out_c=out_c,
                    wqk8_sb=wqk8_sb, wv8_sb=wv8_sb, dwv8_sb=dwv8_sb,
                    wp8_sb=wp8_sb, dwp8_sb=dwp8_sb,
                    tri_sb=tri_sb, sel_sb=sel_sb,
                    bqk_sb=bqk_sb, bout_sb=bout_sb,
                )

    nc.finalize()
    _prog_cache[key] = nc
    return nc


def _emit_body(nc, tc, mybir, rep, *, xt8, dxt8, vone, out_c, wqk8_sb,
               wv8_sb, dwv8_sb, wp8_sb, dwp8_sb, tri_sb, sel_sb,
               bqk_sb, bout_sb):
    F32 = mybir.dt.float32
    F32R = mybir.dt.float32r
    F8 = mybir.dt.float8e4
    AF = mybir.ActivationFunctionType
    MUL = mybir.AluOpType.mult
    SUB = mybir.AluOpType.subtract
    ADD = mybir.AluOpType.add
    DR = mybir.MatmulPerfMode.DoubleRow
    R = f"r{rep}"

    QK_SCALE = AS / (AX * AW)     # PSUM qk -> fp8 tile scale
    V_SCALE = 1.0 / (AX * AW)     # PSUM v -> f32r v_sb
    EXP_SCALE = 0.125 / (AS * AS)  # scores PSUM -> exp
    OUT_SCALE = 1.0 / (AY * AW)   # proj PSUM -> out

    with ExitStack() as persist:
        stP = persist.enter_context(tc.tile_pool(name=f"stP{R}", bufs=1))
        dpool = persist.enter_context(
            tc.tile_pool(name=f"dram{R}", bufs=1, space="DRAM"))
        if VARIANT["scores_nodr"]:
            # plain fp8 scores (K=64): [128=(hh,d), m, t] like v1's qk_sb
            q2_sb = stP.tile([128, 2, T], F8, name=f"q2_sb{R}")
            k2_sb = stP.tile([128, 2, T], F8, name=f"k2_sb{R}")
        else:
            # packed fp8 q/k for DoubleRow scores: [32, head, slot, t],
            # contraction d = 2p + slot
            q2_sb = stP.tile([32, 4, 2, T], F8, name=f"q2_sb{R}")
            k2_sb = stP.tile([32, 4, 2, T], F8, name=f"k2_sb{R}")
        # V natural f32r, 65-stride per head (65th col = ones)
        v_sb = stP.tile([128, NST, 260], F32R, name=f"v_sb{R}")
        # y raw + denominator row (partition 64), per t-chunk; blocks = head
        yraw_q = [
            stP.tile([65, 4, 512], F32R, name=f"yraw{R}_{q}")
            for q in range(NCH)
        ]
        # fp8 y8 (rows 0-255) + dy8 (rows 256-511) packed per chunk
        if VARIANT["ag2"]:
            y_in_q = [
                dpool.tile([1024, 512], F8, name=f"y_in{R}_{q}")
                for q in range(2)
            ]
            y_full_q = [
                dpool.tile([4096, 512], F8, name=f"y_full{R}_{q}")
                for q in range(2)
            ]
        else:
            y_in_q = [
                dpool.tile([512, 512], F8, name=f"y_in{R}_{q}")
                for q in range(NCH)
            ]
            y_full_q = [
                dpool.tile([2048, 512], F8, name=f"y_full{R}_{q}")
                for q in range(NCH)
            ]

        with (
            tc.tile_pool(name=f"stAB{R}", bufs=1) as stAB,
            tc.tile_pool(name=f"psA{R}", bufs=1, space="PSUM") as psA,
        ):
            sy_ctx = ExitStack()
            psS = sy_ctx.enter_context(
                tc.tile_pool(name=f"psS{R}", bufs=1, space="PSUM"))
            psY = sy_ctx.enter_context(
                tc.tile_pool(name=f"psY{R}", bufs=1, space="PSUM"))
            vview = v_sb[:].rearrange("p t (h x) -> p t h x", h=4)
            nc.scalar.dma_start(vview[:, :, :, 64:65], vone[:])
            xt_t, dxt_t = [], []
            for n in range(NCH):
                xtile = stAB.tile([128, NKC, 512], F8, tag="xt", bufs=3,
                                  name=f"xt_t{R}_{n}")
                nc.sync.dma_start(xtile[:], xt8[:, :, n * 512:(n + 1) * 512])
                xt_t.append(xtile)
                dxtile = stAB.tile([128, NKC, 512], F8, tag="dxt", bufs=3,
                                   name=f"dxt_t{R}_{n}")
                nc.sync.dma_start(dxtile[:], dxt8[:, :, n * 512:(n + 1) * 512])
                dxt_t.append(dxtile)

            def emit_norm_ag(n, yraw, r4):
                tmp = stAB.tile([64, 4, 512], F32, tag="tmp", bufs=2,
                                name=f"tmp{R}_{n}")
                for h in range(4):
                    rb = psA.tile([64, 512], F32, tag="pA", bufs=2,
                                  name=f"rb{R}_{n}_{h}")
                    nc.tensor.matmul(
                        rb[:], sel_sb[0:4, h, :], r4[:],
                        start=True, stop=True,
                    )
                    nc.vector.tensor_tensor(
                        tmp[:, h, :], yraw[0:64, h, :], rb[:], MUL,
                    )
                y8t = stAB.tile([64, 4, 512], F8, tag="y8", bufs=2,
                                name=f"y8{R}_{n}")
                nc.vector.tensor_copy(y8t[:], tmp[:])
                dy8t = stAB.tile([64, 4, 512], F8, tag="dy8", bufs=2,
                                 name=f"dy8{R}_{n}")
                nc.vector.tensor_tensor(dy8t[:], tmp[:], y8t[:], SUB)
                if VARIANT["ag2"]:
                    tin = y_in_q[n // 2]
                    b0 = 512 * (n % 2)
                else:
                    tin = y_in_q[n]
                    b0 = 0
                nc.scalar.dma_start(
                    tin[b0:b0 + 256, :].rearrange("(h p) u -> p h u", p=64),
                    y8t[:])
                nc.scalar.dma_start(
                    tin[b0 + 256:b0 + 512, :].rearrange(
                        "(h p) u -> p h u", p=64),
                    dy8t[:])
                if not VARIANT["no_ag"] and (
                        not VARIANT["ag2"] or n % 2 == 1):
                    nc.gpsimd.collective_compute(
                        "AllGather",
                        mybir.AluOpType.bypass,
                        replica_groups=[[0, 1, 2, 3], [4, 5, 6, 7]],
                        ins=[tin[:]],
                        outs=[y_full_q[n // 2 if VARIANT["ag2"] else n][:]],
                    )

            pending = None
            for n in range(NCH):
                    yraw = yraw_q[n]
                    # --- QKV q/k part: 2-pass fp8 DoubleRow ---
                    for m in range(4):
                        ps = psA.tile([128, 512], F32, tag="pA", bufs=2,
                                      name=f"qkvps{R}_{n}_{m}")
                        for xt_pass, first, last in (
                            (xt_t[n], True, False),
                            (dxt_t[n], False, True),
                        ):
                            if VARIANT["big_nodr"]:
                                for kc in range(8):
                                    nc.tensor.matmul(
                                        ps[:],
                                        wqk8_sb[:, kc,
                                                m * 128:(m + 1) * 128],
                                        xt_pass[:, kc, :],
                                        start=(first and kc == 0),
                                        stop=(last and kc == 7),
                                    )
                            else:
                                for kcp in range(4):
                                    nc.tensor.matmul(
                                        ps[:],
                                        wqk8_sb[:, 2 * kcp:2 * kcp + 2,
                                                m * 128:(m + 1) * 128],
                                        xt_pass[:, 2 * kcp:2 * kcp + 2, :],
                                        start=(first and kcp == 0),
                                        stop=(last and kcp == 3),
                                        perf_mode=DR,
                                    )
                        if VARIANT["scores_nodr"]:
                            dst = q2_sb if m < 2 else k2_sb
                            tgt = dst[:, m % 2, n * 512:(n + 1) * 512]
                            if bqk_sb is not None:
                                nc.vector.tensor_scalar(
                                    tgt, ps[:], QK_SCALE,
                                    bqk_sb[:, m:m + 1], MUL, ADD,
                                )
                            else:
                                nc.vector.tensor_scalar_mul(
                                    tgt, ps[:], QK_SCALE)
                            continue
                        qk8t = stAB.tile(
                            [128, 512], F8, tag="qk8",
                            bufs=4 if VARIANT["bufs4"] else 3,
                            name=f"qk8{R}_{n}_{m}")
                        if bqk_sb is not None:
                            nc.vector.tensor_scalar(
                                qk8t[:], ps[:], QK_SCALE,
                                bqk_sb[:, m:m + 1], MUL, ADD,
                            )
                        else:
                            nc.vector.tensor_scalar_mul(
                                qk8t[:], ps[:], QK_SCALE)
                        # repack head halves -> [32, h, slot, t] with the
                        # d = 2p + slot packing (flat DMA orders match, so
                        # one dma_start per head)
                        dst = q2_sb if m < 2 else k2_sb
                        for hh in range(2):
                            h = 2 * (m % 2) + hh
                            nc.sync.dma_start(
                                dst[:, h, :, n * 512:(n + 1) * 512],
                                qk8t[64 * hh:64 * hh + 64, :],
                            )
                    # --- V: 3-pass fp8 DoubleRow ---
                    for tt in range(4):
                        st = 4 * n + tt
                        psv = psA.tile([128, 512], F32, tag="pA", bufs=2,
                                       name=f"vps{R}_{st}")
                        passes = (
                            (xt_t[n], wv8_sb, True, False),
                            (xt_t[n], dwv8_sb, False, False),
                            (dxt_t[n], wv8_sb, False, True),
                        )
                        for xa, wa, first, last in passes:
                            if VARIANT["big_nodr"]:
                                for kc in range(8):
                                    nc.tensor.matmul(
                                        psv[:, 0:256],
                                        xa[:, kc, tt * 128:(tt + 1) * 128],
                                        wa[:, kc, :],
                                        start=(first and kc == 0),
                                        stop=(last and kc == 7),
                                    )
                            else:
                                for kcp in range(4):
                                    nc.tensor.matmul(
                                        psv[:, 0:256],
                                        xa[:, 2 * kcp:2 * kcp + 2,
                                           tt * 128:(tt + 1) * 128],
                                        wa[:, 2 * kcp:2 * kcp + 2, :],
                                        start=(first and kcp == 0),
                                        stop=(last and kcp == 3),
                                        perf_mode=DR,
                                    )
                        nc.vector.tensor_scalar_mul(
                            v_sb[:, st, :].rearrange(
                                "p (h x) -> p h x", h=4)[:, :, 0:64],
                            psv[:, 0:256].rearrange("p (h x) -> p h x", h=4),
                            V_SCALE,
                        )

                    if pending is not None:
                        emit_norm_ag(*pending)
                        pending = None

                    n_st = 4 * n + 4
                    for p in range(2):
                        ype = psY.tile([65, 512], F32, tag="ye", bufs=1,
                                       name=f"ype{R}_{n}_{p}")
                        ypo = psY.tile([65, 512], F32, tag="yo", bufs=1,
                                       name=f"ypo{R}_{n}_{p}")
                        def emit_sc(st):
                            diag = st - 4 * n
                            toff = 128 * diag if diag >= 0 else 0
                            scp = psS.tile([128, 1024], F32, tag="sc", bufs=2,
                                           name=f"scp{R}_{n}_{p}_{st}")
                            es = stAB.tile(
                                [128, 1024], F32R, tag="es",
                                bufs=4 if VARIANT["bufs4"] else 3,
                                name=f"es{R}_{n}_{p}_{st}")
                            for hp in range(2):
                                h = 2 * p + hp
                                if VARIANT["scores_nodr"]:
                                    nc.tensor.matmul(
                                        scp[:, hp * 512 + toff:
                                            (hp + 1) * 512],
                                        k2_sb[64 * hp:64 * hp + 64, p,
                                              st * 128:(st + 1) * 128],
                                        q2_sb[64 * hp:64 * hp + 64, p,
                                              n * 512 + toff:(n + 1) * 512],
                                        start=True, stop=True,
                                    )
                                else:
                                    nc.tensor.matmul(
                                        scp[:, hp * 512 + toff:
                                            (hp + 1) * 512],
                                        k2_sb[:, h, :,
                                              st * 128:(st + 1) * 128],
                                        q2_sb[:, h, :,
                                              n * 512 + toff:(n + 1) * 512],
                                        start=True, stop=True,
                                        perf_mode=DR,
                                    )
                            if diag < 0:
                                nc.scalar.activation(
                                    es[:], scp[:], AF.Exp, scale=EXP_SCALE
                                )
                            else:
                                toff_ = toff
                                esv = es[:].rearrange(
                                    "p (hp u) -> p hp u", hp=2)
                                scv = scp[:].rearrange(
                                    "p (hp u) -> p hp u", hp=2)
                                nc.scalar.activation(
                                    esv[:, :, toff_:512],
                                    scv[:, :, toff_:512],
                                    AF.Exp, scale=EXP_SCALE,
                                )
                                for hp in range(2):
                                    nc.vector.tensor_tensor(
                                        es[:, hp * 512 + toff_:
                                           hp * 512 + toff_ + 128],
                                        es[:, hp * 512 + toff_:
                                           hp * 512 + toff_ + 128],
                                        tri_sb[:], MUL,
                                    )
                            return (st, toff, es)

                        def emit_av(item):
                            st, toff, es = item
                            for hp, yp in ((0, ype), (1, ypo)):
                                h = 2 * p + hp
                                nc.tensor.matmul(
                                    yp[:, toff:512],
                                    v_sb[:, st, 65 * h:65 * h + 65],
                                    es[:, hp * 512 + toff:(hp + 1) * 512],
                                    start=(st == 0), stop=(st == n_st - 1),
                                )

                        if VARIANT["swpipe"]:
                            # issue scores(st+1) before att@V(st) so the PE
                            # queue head never waits on exp(st) with ready
                            # score work behind it
                            prev = None
                            for st in range(n_st):
                                cur = emit_sc(st)
                                if prev is not None:
                                    emit_av(prev)
                                prev = cur
                            emit_av(prev)
                        else:
                            for st in range(n_st):
                                emit_av(emit_sc(st))
                        for hp, yp in ((0, ype), (1, ypo)):
                            h = 2 * p + hp
                            nc.vector.tensor_copy(yraw[:, h, :], yp[:])

                    den4 = stAB.tile([4, 512], F32R, tag="den4", bufs=2,
                                     name=f"den4{R}_{n}")
                    nc.scalar.dma_start(den4[:], yraw[64:65, :, :])
                    rf = stAB.tile([4, 512], F32, tag="rf", bufs=2,
                                   name=f"rf{R}_{n}")
                    nc.vector.reciprocal_approx_fast(
                        rf[:], den4[:].bitcast(F32))
                    r4 = stAB.tile([4, 512], F32R, tag="r4", bufs=2,
                                   name=f"r4{R}_{n}")
                    nc.vector.tensor_scalar_mul(r4[:], rf[:], AY)
                    pending = (n, yraw, r4)

            # free the attention score/accumulator banks (6), keep psA
            # open for the final normalize; proj 0-2 draws only from the
            # freed space, so it is NOT gated on the last recip chain.
            sy_ctx.close()

            def emit_proj(q, psP):
                pp0 = psP.tile([128, 512], F32, tag="pp0", bufs=2,
                               name=f"pp0{R}_{q}")
                pp1 = psP.tile([128, 512], F32, tag="pp1", bufs=2,
                               name=f"pp1{R}_{q}")
                for kcp in range(4):
                    # [p, slot j, w (y8 vs dy8), t]; one DMA per rank block
                    ydf = stAB.tile([128, 2, 2, 512], F8, tag="yf", bufs=4,
                                    name=f"ydf{R}_{q}_{kcp}")
                    if q < 3:
                        dma_eng = nc.sync
                    else:
                        dma_eng = nc.sync if kcp % 2 == 0 else nc.scalar
                    for w in range(2):
                        if VARIANT["ag2"]:
                            ysrc = y_full_q[q // 2]
                            base = 1024 * kcp + 512 * (q % 2) + 256 * w
                        else:
                            ysrc = y_full_q[q]
                            base = 512 * kcp + 256 * w
                        dma_eng.dma_start(
                            ydf[:, :, w, :],
                            ysrc[base:base + 256, :].rearrange(
                                "(j p) u -> p j u", j=2),
                        )
                    yf = ydf[:, :, 0, :]
                    dyf = ydf[:, :, 1, :]
                    for m2, pp in ((0, pp0), (1, pp1)):
                        wsl = slice(m2 * 128, (m2 + 1) * 128)
                        if VARIANT["big_nodr"]:
                            for j in range(2):
                                kc = 2 * kcp + j
                                nc.tensor.matmul(
                                    pp[:], wp8_sb[:, kc, wsl],
                                    ydf[:, j, 0, :],
                                    start=(kcp == 0 and j == 0), stop=False,
                                )
                                nc.tensor.matmul(
                                    pp[:], dwp8_sb[:, kc, wsl],
                                    ydf[:, j, 0, :],
                                    start=False, stop=False,
                                )
                                nc.tensor.matmul(
                                    pp[:], wp8_sb[:, kc, wsl],
                                    ydf[:, j, 1, :],
                                    start=False,
                                    stop=(kcp == 3 and j == 1),
                                )
                        else:
                            ksl = slice(2 * kcp, 2 * kcp + 2)
                            nc.tensor.matmul(
                                pp[:], wp8_sb[:, ksl, wsl], yf,
                                start=(kcp == 0), stop=False, perf_mode=DR,
                            )
                            nc.tensor.matmul(
                                pp[:], dwp8_sb[:, ksl, wsl], yf,
                                start=False, stop=False, perf_mode=DR,
                            )
                            nc.tensor.matmul(
                                pp[:], wp8_sb[:, ksl, wsl], dyf,
                                start=False, stop=(kcp == 3), perf_mode=DR,
                            )
                out_sb = stAB.tile([128, 2, 512], F32, tag="out_sb", bufs=2,
                                   name=f"out_sb{R}_{q}")
                for m2, pp in ((0, pp0), (1, pp1)):
                    if bout_sb is not None:
                        nc.vector.tensor_scalar(
                            out_sb[:, m2, :], pp[:], OUT_SCALE,
                            bout_sb[:, m2:m2 + 1], MUL, ADD,
                        )
                    else:
                        nc.vector.tensor_scalar_mul(
                            out_sb[:, m2, :], pp[:], OUT_SCALE)
                nc.sync.dma_start(
                    out_c[:, q * 512:(q + 1) * 512].rearrange(
                        "(m p) t -> p m t", p=128),
                    out_sb[:],
                )

            with tc.tile_pool(name=f"psP{R}", bufs=1, space="PSUM") as psP:
                if VARIANT["no_proj"]:
                    emit_norm_ag(*pending)
                    junk = stAB.tile([128, 2, 512], F32, tag="out_sb", bufs=2,
                                     name=f"junk{R}")
                    nc.vector.memset(junk[:], 0.0)
                    for q in range(NCH):
                        nc.sync.dma_start(
                            out_c[:, q * 512:(q + 1) * 512].rearrange(
                                "(m p) t -> p m t", p=128),
                            junk[:],
                        )
                elif VARIANT["ag2"]:
                    emit_proj(0, psP)
                    emit_proj(1, psP)
                    emit_norm_ag(*pending)
                    emit_proj(2, psP)
                    emit_proj(3, psP)
                else:
                    for q in range(3):
                        emit_proj(q, psP)
                    emit_norm_ag(*pending)
                    emit_proj(3, psP)


def _chunked(a):
    """(C, X) -> [128, C/128, X] contraction-chunked layout."""
    c, x = a.shape
    return np.ascontiguousarray(
        a.reshape(c // 128, 128, x).transpose(1, 0, 2)
    )


def _q8(a):
    return np.asarray(a, dtype=E4NP)


def make_in_maps(x, w_attn, b_attn, w_proj, b_proj):
    x = np.asarray(x, dtype=np.float32)
    w_attn = np.asarray(w_attn, dtype=np.float32)
    b_attn = np.asarray(b_attn, dtype=np.float32)
    w_proj = np.asarray(w_proj, dtype=np.float32)
    b_proj = np.asarray(b_proj, dtype=np.float32)

    qk_bias = bool(np.any(b_attn[: 2 * C] != 0))
    b_out_full = b_attn[2 * C:] @ w_proj + b_proj  # V bias folds through
    out_bias = bool(np.any(b_out_full != 0))

    tri_np = np.triu(np.ones((128, 128), np.float32))
    vone_np = np.ones((128, NST, 4, 1), np.float32)
    sel_np = np.zeros((16, 16, 64), np.float32)
    for i in range(16):
        sel_np[i, i, :] = 1.0

    xt8_g, dxt8_g = [], []
    for g in range(B):
        X = np.ascontiguousarray(x[g].T) * AX
        x8 = _q8(X)
        dx8 = _q8(X - x8.astype(np.float32))
        xt8_g.append(_chunked(x8.astype(np.float32)).astype(E4NP))
        dxt8_g.append(_chunked(dx8.astype(np.float32)).astype(E4NP))

    in_maps = []
    for core in range(NCORES):
        g, r = core // 4, core % 4
        h0 = r * HL
        qcols = slice(h0 * D, (h0 + HL) * D)
        kcols = slice(C + h0 * D, C + (h0 + HL) * D)
        vcols = slice(2 * C + h0 * D, 2 * C + (h0 + HL) * D)
        wqk_s = np.concatenate(
            [w_attn[:, qcols], w_attn[:, kcols]], axis=1) * AW
        wqk8_np = _chunked(_q8(wqk_s).astype(np.float32)).astype(E4NP)
        wv_s = np.ascontiguousarray(w_attn[:, vcols]) * AW
        wv8 = _q8(wv_s)
        dwv8 = _q8(wv_s - wv8.astype(np.float32))
        wp_s = np.ascontiguousarray(w_proj[:, 256 * r: 256 * (r + 1)]) * AW
        wp8 = _q8(wp_s)
        dwp8 = _q8(wp_s - wp8.astype(np.float32))
        m = {
            "xt8": xt8_g[g],
            "dxt8": dxt8_g[g],
            "wqk8": wqk8_np,
            "wv8": _chunked(wv8.astype(np.float32)).astype(E4NP),
            "dwv8": _chunked(dwv8.astype(np.float32)).astype(E4NP),
            "wp8": _chunked(wp8.astype(np.float32)).astype(E4NP),
            "dwp8": _chunked(dwp8.astype(np.float32)).astype(E4NP),
            "tri": tri_np,
            "vone": vone_np,
            "sel": sel_np,
        }
        if qk_bias:
            bq = np.concatenate([b_attn[qcols], b_attn[kcols]]) * AS  # (512,)
            m["bqk"] = np.ascontiguousarray(
                bq.reshape(4, 128).T.astype(np.float32))
        if out_bias:
            bo = b_out_full[256 * r: 256 * (r + 1)]
            m["bout"] = np.ascontiguousarray(
                bo.reshape(2, 128).T.astype(np.float32))
        in_maps.append(m)
    return in_maps, qk_bias, out_bias


def assemble_output(results):
    out = np.empty((B, T, C), dtype=np.float32)
    for core in range(NCORES):
        g, r = core // 4, core % 4
        out[g][:, 256 * r: 256 * (r + 1)] = results[core]["out_c"].T
    return out


def kernel(x, w_attn, b_attn, w_proj, b_proj):
    from concourse.bass_utils import run_bass_kernel_spmd

    in_maps, qk_bias, out_bias = make_in_maps(
        x, w_attn, b_attn, w_proj, b_proj)
    nc = build_program(reps=1, qk_bias=qk_bias, out_bias=out_bias)
    res = run_bass_kernel_spmd(nc, in_maps, list(range(NCORES)))
    return assemble_output(res.results)


# revision 26
# speedup vs baseline: 1.0520x; 1.0017x over previous
"""Causal self-attention (B=2, T=2048, C=1024, H=16, D=64) on 8 TRN2 cores.

Sharding: batch across 2 groups of 4 cores; 4 heads per core within a group
(Megatron column-parallel QKV). After attention, AllGather the per-head
outputs within each group (fp8 value+residual pair), then column-parallel
c_proj (each core computes 256 output columns for all T).

fp8 strategy (all scale factors are exact powers of two, folded into
existing copies / the exp scale, so descale costs nothing):
  X' = x^T*AX quantized to fp8e4m3 host-side as x8 + residual dx8.
  W' = w*AW quantized host-side (wqk8 single; wv8+dwv8, wp8+dwp8 pairs).
  QK  = (x8 + dx8)@wqk8          2-pass DoubleRow (w-quant error is washed
                                 out by softmax; x-residual is not, so kept)
  V   = x8@wv8 + x8@dwv8 + dx8@wv8   3-pass DoubleRow (~exact: only the
                                 2nd-order dx8@dwv8 term is dropped)
  S   = q8^T k8 plain fp8 over D=64 (q,k requantized fp8*AS; softmax
                                 washes the quant error; DoubleRow scores
                                 measured slower on HW due to repack DMAs)
  att@V in fp32r (fp8 attention weights would cost ~1.4e-2 error alone).
  y -> y8 + dy8 fp8 pair (~exact); two packed [1024,512] AllGathers (chunk
  pairs) -- fewer group-sync points measured faster than four.
  proj = y8@wp8 + y8@dwp8 + dy8@wp8  3-pass DoubleRow (~exact)

Measured end-to-end rel err ~1.3e-2 vs the 2e-2 gate (numpy model 1.37e-2).

Per-core PE work drops from ~278K cycle-rows (all fp32r) to ~200K:
QKV 32.8K + V 24.6K + S 34.8K + att@V 69.6K(+diag) + norm 8.2K + proj 24.6K.

QKV (stage A) and attention (stage B) are emitted interleaved per t-chunk;
each chunk's normalize + AllGather is deferred by one chunk and issued
mid-compute so the first three AllGathers hide under attention. The
score/accumulator PSUM pools close right after attention so proj quarters
0-2 run before the final normalize+AllGather.

Output per core: out_c [256, 2048] = out^T columns slice; host reassembles.
"""

import sys

sys.path.insert(0, "/opt/trn_rl_repo")

from contextlib import ExitStack

import numpy as np
import ml_dtypes

B, T, C, H, D = 2, 2048, 1024, 16, 64
NCORES = 8
HL = 4   # heads per core
NKC = 8  # contraction chunks (C / 128)
NCH = 4  # t chunks (T / 512)
NST = 16  # s tiles (T / 128)

AX = 8.0    # x fp8 scale
AW = 256.0  # weight fp8 scale
AS = 16.0   # q/k score-input fp8 scale
AY = 32.0   # y fp8 scale

E4NP = ml_dtypes.float8_e4m3

_prog_cache = {}

# ablation flags for perf experiments (not used in production path)
VARIANT = {"no_ag": False, "no_proj": False, "scores_nodr": True,
           "ag_sync": False, "bufs4": False, "big_nodr": False,
           "ag2": True, "swpipe": True}


def build_program(reps=1, qk_bias=False, out_bias=False):
    key = (reps, qk_bias, out_bias, tuple(sorted(VARIANT.items())))
    if key in _prog_cache:
        return _prog_cache[key]

    from concourse import bacc, mybir
    import concourse.tile as tile

    F32 = mybir.dt.float32
    F32R = mybir.dt.float32r
    F8 = mybir.dt.float8e4

    nc = bacc.Bacc(num_devices=NCORES)

    xt8 = nc.declare_dram_parameter("xt8", [128, NKC, T], F8, isOutput=False)
    dxt8 = nc.declare_dram_parameter("dxt8", [128, NKC, T], F8, isOutput=False)
    wqk8 = nc.declare_dram_parameter("wqk8", [128, NKC, 512], F8, isOutput=False)
    wv8 = nc.declare_dram_parameter("wv8", [128, NKC, 256], F8, isOutput=False)
    dwv8 = nc.declare_dram_parameter("dwv8", [128, NKC, 256], F8, isOutput=False)
    wp8 = nc.declare_dram_parameter("wp8", [128, NKC, 256], F8, isOutput=False)
    dwp8 = nc.declare_dram_parameter("dwp8", [128, NKC, 256], F8, isOutput=False)
    tri = nc.declare_dram_parameter("tri", [128, 128], F32R, isOutput=False)
    vone = nc.declare_dram_parameter("vone", [128, NST, 4, 1], F32R, isOutput=False)
    sel = nc.declare_dram_parameter("sel", [16, 16, 64], F32R, isOutput=False)
    if qk_bias:
        bqk = nc.declare_dram_parameter("bqk", [128, 4], F32, isOutput=False)
    if out_bias:
        bout = nc.declare_dram_parameter("bout", [128, 2], F32, isOutput=False)
    out_c = nc.declare_dram_parameter("out_c", [256, T], F32, isOutput=True)

    with tile.TileContext(nc) as tc:
        with ExitStack() as outer:
            const = outer.enter_context(tc.tile_pool(name="const", bufs=1))
            wqk8_sb = const.tile([128, NKC, 512], F8)
            wv8_sb = const.tile([128, NKC, 256], F8)
            dwv8_sb = const.tile([128, NKC, 256], F8)
            wp8_sb = const.tile([128, NKC, 256], F8)
            dwp8_sb = const.tile([128, NKC, 256], F8)
            tri_sb = const.tile([128, 128], F32R)
            sel_sb = const.tile([16, 16, 64], F32R)
            nc.scalar.dma_start(wqk8_sb[:], wqk8[:])
            nc.scalar.dma_start(wv8_sb[:], wv8[:])
            nc.scalar.dma_start(dwv8_sb[:], dwv8[:])
            nc.scalar.dma_start(wp8_sb[:], wp8[:])
            nc.scalar.dma_start(dwp8_sb[:], dwp8[:])
            nc.scalar.dma_start(tri_sb[:], tri[:])
            nc.scalar.dma_start(sel_sb[:], sel[:])
            bqk_sb = bout_sb = None
            if qk_bias:
                bqk_sb = const.tile([128, 4], F32)
                nc.scalar.dma_start(bqk_sb[:], bqk[:])
            if out_bias:
                bout_sb = const.tile([128, 2], F32)
                nc.scalar.dma_start(bout_sb[:], bout[:])

            for rep in range(reps):
                _emit_body(
                    nc, tc, mybir, rep,
                    xt8=xt8, dxt8=dxt8, vone=vone, out_c=out_c,
                    wqk8_sb=wqk8_sb, wv8_sb=wv8_sb, dwv8_sb=dwv8_sb,
                    wp8_sb=wp8_sb, dwp8_sb=dwp8_sb,
                    tri_sb=tri_sb, sel_sb=sel_sb,
                    bqk_sb=bqk_sb, bout_sb=bout_sb,
                )

    nc.finalize()
    _prog_cache[key] = nc
    return nc


def _emit_body(nc, tc, mybir, rep, *, xt8, dxt8, vone, out_c, wqk8_sb,
               wv8_sb, dwv8_sb, wp8_sb, dwp8_sb, tri_sb, sel_sb,
               bqk_sb, bout_sb):
    F32 = mybir.dt.float32
    F32R = mybir.dt.float32r
    F8 = mybir.dt.float8e4
    AF = mybir.ActivationFunctionType
    MUL = mybir.AluOpType.mult
    SUB = mybir.AluOpType.subtract
    ADD = mybir.AluOpType.add
    DR = mybir.MatmulPerfMode.DoubleRow
    R = f"r{rep}"

    QK_SCALE = AS / (AX * AW)     # PSUM qk -> fp8 tile scale
    V_SCALE = 1.0 / (AX * AW)     # PSUM v -> f32r v_sb
    EXP_SCALE = 0.125 / (AS * AS)  # scores PSUM -> exp
    OUT_SCALE = 1.0 / (AY * AW)   # proj PSUM -> out

    with ExitStack() as persist:
        stP = persist.enter_context(tc.tile_pool(name=f"stP{R}", bufs=1))
        dpool = persist.enter_context(
            tc.tile_pool(name=f"dram{R}", bufs=1, space="DRAM"))
        if VARIANT["scores_nodr"]:
            # plain fp8 scores (K=64): [128=(hh,d), m, t] like v1's qk_sb
            q2_sb = stP.tile([128, 2, T], F8, name=f"q2_sb{R}")
            k2_sb = stP.tile([128, 2, T], F8, name=f"k2_sb{R}")
        else:
            # packed fp8 q/k for DoubleRow scores: [32, head, slot, t],
            # contraction d = 2p + slot
            q2_sb = stP.tile([32, 4, 2, T], F8, name=f"q2_sb{R}")
            k2_sb = stP.tile([32, 4, 2, T], F8, name=f"k2_sb{R}")
        # V natural f32r, 65-stride per head (65th col = ones)
        v_sb = stP.tile([128, NST, 260], F32R, name=f"v_sb{R}")
        # y raw + denominator row (partition 64), per t-chunk; blocks = head
        yraw_q = [
            stP.tile([65, 4, 512], F32R, name=f"yraw{R}_{q}")
            for q in range(NCH)
        ]
        # fp8 y8 (rows 0-255) + dy8 (rows 256-511) packed per chunk
        if VARIANT["ag2"]:
            y_in_q = [
                dpool.tile([1024, 512], F8, name=f"y_in{R}_{q}")
                for q in range(2)
            ]
            y_full_q = [
                dpool.tile([4096, 512], F8, name=f"y_full{R}_{q}")
                for q in range(2)
            ]
        else:
            y_in_q = [
                dpool.tile([512, 512], F8, name=f"y_in{R}_{q}")
                for q in range(NCH)
            ]
            y_full_q = [
                dpool.tile([2048, 512], F8, name=f"y_full{R}_{q}")
                for q in range(NCH)
            ]

        with (
            tc.tile_pool(name=f"stAB{R}", bufs=1) as stAB,
            tc.tile_pool(name=f"psA{R}", bufs=1, space="PSUM") as psA,
        ):
            sy_ctx = ExitStack()
            psS = sy_ctx.enter_context(
                tc.tile_pool(name=f"psS{R}", bufs=1, space="PSUM"))
            psY = sy_ctx.enter_context(
                tc.tile_pool(name=f"psY{R}", bufs=1, space="PSUM"))
            vview = v_sb[:].rearrange("p t (h x) -> p t h x", h=4)
            nc.scalar.dma_start(vview[:, :, :, 64:65], vone[:])
            xt_t, dxt_t = [], []
            for n in range(NCH):
                xtile = stAB.tile([128, NKC, 512], F8, tag="xt", bufs=3,
                                  name=f"xt_t{R}_{n}")
                nc.sync.dma_start(xtile[:], xt8[:, :, n * 512:(n + 1) * 512])
                xt_t.append(xtile)
                dxtile = stAB.tile([128, NKC, 512], F8, tag="dxt", bufs=3,
                                   name=f"dxt_t{R}_{n}")
                nc.sync.dma_start(dxtile[:], dxt8[:, :, n * 512:(n + 1) * 512])
                dxt_t.append(dxtile)

            def emit_norm_ag(n, yraw, r4):
                tmp = stAB.tile([64, 4, 512], F32, tag="tmp", bufs=2,
                                name=f"tmp{R}_{n}")
                for h in range(4):
                    rb = psA.tile([64, 512], F32, tag="pA", bufs=2,
                                  name=f"rb{R}_{n}_{h}")
                    nc.tensor.matmul(
                        rb[:], sel_sb[0:4, h, :], r4[:],
                        start=True, stop=True,
                    )
                    nc.vector.tensor_tensor(
                        tmp[:, h, :], yraw[0:64, h, :], rb[:], MUL,
                    )
                y8t = stAB.tile([64, 4, 512], F8, tag="y8", bufs=2,
                                name=f"y8{R}_{n}")
                nc.vector.tensor_copy(y8t[:], tmp[:])
                dy8t = stAB.tile([64, 4, 512], F8, tag="dy8", bufs=2,
                                 name=f"dy8{R}_{n}")
                nc.vector.tensor_tensor(dy8t[:], tmp[:], y8t[:], SUB)
                if VARIANT["ag2"]:
                    tin = y_in_q[n // 2]
                    b0 = 512 * (n % 2)
                else:
                    tin = y_in_q[n]
                    b0 = 0
                nc.scalar.dma_start(
                    tin[b0:b0 + 256, :].rearrange("(h p) u -> p h u", p=64),
                    y8t[:])
                nc.scalar.dma_start(
                    tin[b0 + 256:b0 + 512, :].rearrange(
                        "(h p) u -> p h u", p=64),
                    dy8t[:])
                if not VARIANT["no_ag"] and (
                        not VARIANT["ag2"] or n % 2 == 1):
                    nc.gpsimd.collective_compute(
                        "AllGather",
                        mybir.AluOpType.bypass,
                        replica_groups=[[0, 1, 2, 3], [4, 5, 6, 7]],
                        ins=[tin[:]],
                        outs=[y_full_q[n // 2 if VARIANT["ag2"] else n][:]],
                    )

            pending = None
            for n in range(NCH):
                    yraw = yraw_q[n]
                    # --- QKV q/k part: 2-pass fp8 DoubleRow ---
                    for m in range(4):
                        ps = psA.tile([128, 512], F32, tag="pA", bufs=2,
                                      name=f"qkvps{R}_{n}_{m}")
                        for xt_pass, first, last in (
                            (xt_t[n], True, False),
                            (dxt_t[n], False, True),
                        ):
                            if VARIANT["big_nodr"]:
                                for kc in range(8):
                                    nc.tensor.matmul(
                                        ps[:],
                                        wqk8_sb[:, kc,
                                                m * 128:(m + 1) * 128],
                                        xt_pass[:, kc, :],
                                        start=(first and kc == 0),
                                        stop=(last and kc == 7),
                                    )
                            else:
                                for kcp in range(4):
                                    nc.tensor.matmul(
                                        ps[:],
                                        wqk8_sb[:, 2 * kcp:2 * kcp + 2,
                                                m * 128:(m + 1) * 128],
                                        xt_pass[:, 2 * kcp:2 * kcp + 2, :],
                                        start=(first and kcp == 0),
                                        stop=(last and kcp == 3),
                                        perf_mode=DR,
                                    )
                        if VARIANT["scores_nodr"]:
                            dst = q2_sb if m < 2 else k2_sb
                            tgt = dst[:, m % 2, n * 512:(n + 1) * 512]
                            if bqk_sb is not None:
                                nc.vector.tensor_scalar(
                                    tgt, ps[:], QK_SCALE,
                                    bqk_sb[:, m:m + 1], MUL, ADD,
                                )
                            else:
                                nc.vector.tensor_scalar_mul(
                                    tgt, ps[:], QK_SCALE)
                            continue
                        qk8t = stAB.tile(
                            [128, 512], F8, tag="qk8",
                            bufs=4 if VARIANT["bufs4"] else 3,
                            name=f"qk8{R}_{n}_{m}")
                        if bqk_sb is not None:
                            nc.vector.tensor_scalar(
                                qk8t[:], ps[:], QK_SCALE,
                                bqk_sb[:, m:m + 1], MUL, ADD,
                            )
                        else:
                            nc.vector.tensor_scalar_mul(
                                qk8t[:], ps[:], QK_SCALE)
                        # repack head halves -> [32, h, slot, t] with the
                        # d = 2p + slot packing (flat DMA orders match, so
                        # one dma_start per head)
                        dst = q2_sb if m < 2 else k2_sb
                        for hh in range(2):
                            h = 2 * (m % 2) + hh
                            nc.sync.dma_start(
                                dst[:, h, :, n * 512:(n + 1) * 512],
                                qk8t[64 * hh:64 * hh + 64, :],
                            )
                    # --- V: 3-pass fp8 DoubleRow ---
                    for tt in range(4):
                        st = 4 * n + tt
                        psv = psA.tile([128, 512], F32, tag="pA", bufs=2,
                                       name=f"vps{R}_{st}")
                        passes = (
                            (xt_t[n], wv8_sb, True, False),
                            (xt_t[n], dwv8_sb, False, False),
                            (dxt_t[n], wv8_sb, False, True),
                        )
                        for xa, wa, first, last in passes:
                            if VARIANT["big_nodr"]:
                                for kc in range(8):
                                    nc.tensor.matmul(
                                        psv[:, 0:256],
                                        xa[:, kc, tt * 128:(tt + 1) * 128],
                                        wa[:, kc, :],
                                        start=(first and kc == 0),
                                        stop=(last and kc == 7),
                                    )
                            else:
                                for kcp in range(4):
                                    nc.tensor.matmul(
                                        psv[:, 0:256],
                                        xa[:, 2 * kcp:2 * kcp + 2,
                                           tt * 128:(tt + 1) * 128],
                                        wa[:, 2 * kcp:2 * kcp + 2, :],
                                        start=(first and kcp == 0),
                                        stop=(last and kcp == 3),
                                        perf_mode=DR,
                                    )
                        nc.vector.tensor_scalar_mul(
                            v_sb[:, st, :].rearrange(
                                "p (h x) -> p h x", h=4)[:, :, 0:64],
                            psv[:, 0:256].rearrange("p (h x) -> p h x", h=4),
                            V_SCALE,
                        )

                    if pending is not None:
                        emit_norm_ag(*pending)
                        pending = None

                    n_st = 4 * n + 4
                    for p in range(2):
                        ype = psY.tile([65, 512], F32, tag="ye", bufs=1,
                                       name=f"ype{R}_{n}_{p}")
                        ypo = psY.tile([65, 512], F32, tag="yo", bufs=1,
                                       name=f"ypo{R}_{n}_{p}")
                        def emit_sc(st):
                            diag = st - 4 * n
                            toff = 128 * diag if diag >= 0 else 0
                            scp = psS.tile([128, 1024], F32, tag="sc", bufs=2,
                                           name=f"scp{R}_{n}_{p}_{st}")
                            es = stAB.tile(
                                [128, 1024], F32R, tag="es",
                                bufs=4 if VARIANT["bufs4"] else 3,
                                name=f"es{R}_{n}_{p}_{st}")
                            for hp in range(2):
                                h = 2 * p + hp
                                if VARIANT["scores_nodr"]:
                                    nc.tensor.matmul(
                                        scp[:, hp * 512 + toff:
                                            (hp + 1) * 512],
                                        k2_sb[64 * hp:64 * hp + 64, p,
                                              st * 128:(st + 1) * 128],
                                        q2_sb[64 * hp:64 * hp + 64, p,
                                              n * 512 + toff:(n + 1) * 512],
                                        start=True, stop=True,
                                    )
                                else:
                                    nc.tensor.matmul(
                                        scp[:, hp * 512 + toff:
                                            (hp + 1) * 512],
                                        k2_sb[:, h, :,
                                              st * 128:(st + 1) * 128],
                                        q2_sb[:, h, :,
                                              n * 512 + toff:(n + 1) * 512],
                                        start=True, stop=True,
                                        perf_mode=DR,
                                    )
                            if diag < 0:
                                nc.scalar.activation(
                                    es[:], scp[:], AF.Exp, scale=EXP_SCALE
                                )
                            else:
                                toff_ = toff
                                esv = es[:].rearrange(
                                    "p (hp u) -> p hp u", hp=2)
                                scv = scp[:].rearrange(
                                    "p (hp u) -> p hp u", hp=2)
                                nc.scalar.activation(
                                    esv[:, :, toff_:512],
                                    scv[:, :, toff_:512],
                                    AF.Exp, scale=EXP_SCALE,
                                )
                                for hp in range(2):
                                    nc.vector.tensor_tensor(
                                        es[:, hp * 512 + toff_:
                                           hp * 512 + toff_ + 128],
                                        es[:, hp * 512 + toff_:
                                           hp * 512 + toff_ + 128],
                                        tri_sb[:], MUL,
                                    )
                            return (st, toff, es)

                        def emit_av(item):
                            st, toff, es = item
                            for hp, yp in ((0, ype), (1, ypo)):
                                h = 2 * p + hp
                                nc.tensor.matmul(
                                    yp[:, toff:512],
                                    v_sb[:, st, 65 * h:65 * h + 65],
                                    es[:, hp * 512 + toff:(hp + 1) * 512],
                                    start=(st == 0), stop=(st == n_st - 1),
                                )

                        if VARIANT["swpipe"]:
                            # issue scores(st+1) before att@V(st) so the PE
                            # queue head never waits on exp(st) with ready
                            # score work behind it
                            prev = None
                            for st in range(n_st):
                                cur = emit_sc(st)
                                if prev is not None:
                                    emit_av(prev)
                                prev = cur
                            emit_av(prev)
                        else:
                            for st in range(n_st):
                                emit_av(emit_sc(st))
                        for hp, yp in ((0, ype), (1, ypo)):
                            h = 2 * p + hp
                            nc.vector.tensor_copy(yraw[:, h, :], yp[:])

                    den4 = stAB.tile([4, 512], F32R, tag="den4", bufs=2,
                                     name=f"den4{R}_{n}")
                    nc.scalar.dma_start(den4[:], yraw[64:65, :, :])
                    rf = stAB.tile([4, 512], F32, tag="rf", bufs=2,
                                   name=f"rf{R}_{n}")
                    nc.vector.reciprocal_approx_fast(
                        rf[:], den4[:].bitcast(F32))
                    r4 = stAB.tile([4, 512], F32R, tag="r4", bufs=2,
                                   name=f"r4{R}_{n}")
                    nc.vector.tensor_scalar_mul(r4[:], rf[:], AY)
                    pending = (n, yraw, r4)

            # free the attention score/accumulator banks (6), keep psA
            # open for the final normalize; proj 0-2 draws only from the
            # freed space, so it is NOT gated on the last recip chain.
            sy_ctx.close()

            def emit_proj(q, psP):
                pp0 = psP.tile([128, 512], F32, tag="pp0", bufs=2,
                               name=f"pp0{R}_{q}")
                pp1 = psP.tile([128, 512], F32, tag="pp1", bufs=2,
                               name=f"pp1{R}_{q}")
                for kcp in range(4):
                    # [p, slot j, w (y8 vs dy8), t]; one DMA per rank block
                    ydf = stAB.tile([128, 2, 2, 512], F8, tag="yf", bufs=4,
                                    name=f"ydf{R}_{q}_{kcp}")
                    if q < 3:
                        dma_eng = nc.sync
                    else:
                        dma_eng = nc.sync if kcp % 2 == 0 else nc.scalar
                    for w in range(2):
                        if VARIANT["ag2"]:
                            ysrc = y_full_q[q // 2]
                            base = 1024 * kcp + 512 * (q % 2) + 256 * w
                        else:
                            ysrc = y_full_q[q]
                            base = 512 * kcp + 256 * w
                        dma_eng.dma_start(
                            ydf[:, :, w, :],
                            ysrc[base:base + 256, :].rearrange(
                                "(j p) u -> p j u", j=2),
                        )
                    yf = ydf[:, :, 0, :]
                    dyf = ydf[:, :, 1, :]
                    for m2, pp in ((0, pp0), (1, pp1)):
                        wsl = slice(m2 * 128, (m2 + 1) * 128)
                        if VARIANT["big_nodr"]:
                            for j in range(2):
                                kc = 2 * kcp + j
                                nc.tensor.matmul(
                                    pp[:], wp8_sb[:, kc, wsl],
                                    ydf[:, j, 0, :],
                                    start=(kcp == 0 and j == 0), stop=False,
                                )
                                nc.tensor.matmul(
                                    pp[:], dwp8_sb[:, kc, wsl],
                                    ydf[:, j, 0, :],
                                    start=False, stop=False,
                                )
                                nc.tensor.matmul(
                                    pp[:], wp8_sb[:, kc, wsl],
                                    ydf[:, j, 1, :],
                                    start=False,
                                    stop=(kcp == 3 and j == 1),
                                )
                        else:
                            ksl = slice(2 * kcp, 2 * kcp + 2)
                            nc.tensor.matmul(
                                pp[:], wp8_sb[:, ksl, wsl], yf,
                                start=(kcp == 0), stop=False, perf_mode=DR,
                            )
                            nc.tensor.matmul(
                                pp[:], dwp8_sb[:, ksl, wsl], yf,
                                start=False, stop=False, perf_mode=DR,
                            )
                            nc.tensor.matmul(
                                pp[:], wp8_sb[:, ksl, wsl], dyf,
                                start=False, stop=(kcp == 3), perf_mode=DR,
                            )
                out_sb = stAB.tile([128, 2, 512], F32, tag="out_sb", bufs=2,
                                   name=f"out_sb{R}_{q}")
                for m2, pp in ((0, pp0), (1, pp1)):
                    if bout_sb is not None:
                        nc.vector.tensor_scalar(
                            out_sb[:, m2, :], pp[:], OUT_SCALE,
                            bout_sb[:, m2:m2 + 1], MUL, ADD,
                        )
                    else:
                        nc.vector.tensor_scalar_mul(
                            out_sb[:, m2, :], pp[:], OUT_SCALE)
                nc.sync.dma_start(
                    out_c[:, q * 512:(q + 1) * 512].rearrange(
                        "(m p) t -> p m t", p=128),
                    out_sb[:],
                )

            with tc.tile_pool(name=f"psP{R}", bufs=1, space="PSUM") as psP:
                if VARIANT["no_proj"]:
                    emit_norm_ag(*pending)
                    junk = stAB.tile([128, 2, 512], F32, tag="out_sb", bufs=2,
                                     name=f"junk{R}")
                    nc.vector.memset(junk[:], 0.0)
                    for q in range(NCH):
                        nc.sync.dma_start(
                            out_c[:, q * 512:(q + 1) * 512].rearrange(
                                "(m p) t -> p m t", p=128),
                            junk[:],
                        )
                elif VARIANT["ag2"]:
                    # norm3+AG1 first: their PE/DVE work would otherwise
                    # queue behind proj q0/q1, delaying the final gather
                    # whose transfer can hide under all four proj quarters
                    emit_norm_ag(*pending)
                    for q in range(4):
                        emit_proj(q, psP)
                else:
                    for q in range(3):
                        emit_proj(q, psP)
                    emit_norm_ag(*pending)
                    emit_proj(3, psP)


def _chunked(a):
    """(C, X) -> [128, C/128, X] contraction-chunked layout."""
    c, x = a.shape
    return np.ascontiguousarray(
        a.reshape(c // 128, 128, x).transpose(1, 0, 2)
    )


def _q8(a):
    return np.asarray(a, dtype=E4NP)


def make_in_maps(x, w_attn, b_attn, w_proj, b_proj):
    x = np.asarray(x, dtype=np.float32)
    w_attn = np.asarray(w_attn, dtype=np.float32)
    b_attn = np.asarray(b_attn, dtype=np.float32)
    w_proj = np.asarray(w_proj, dtype=np.float32)
    b_proj = np.asarray(b_proj, dtype=np.float32)

    qk_bias = bool(np.any(b_attn[: 2 * C] != 0))
    b_out_full = b_attn[2 * C:] @ w_proj + b_proj  # V bias folds through
    out_bias = bool(np.any(b_out_full != 0))

    tri_np = np.triu(np.ones((128, 128), np.float32))
    vone_np = np.ones((128, NST, 4, 1), np.float32)
    sel_np = np.zeros((16, 16, 64), np.float32)
    for i in range(16):
        sel_np[i, i, :] = 1.0

    xt8_g, dxt8_g = [], []
    for g in range(B):
        X = np.ascontiguousarray(x[g].T) * AX
        x8 = _q8(X)
        dx8 = _q8(X - x8.astype(np.float32))
        xt8_g.append(_chunked(x8.astype(np.float32)).astype(E4NP))
        dxt8_g.append(_chunked(dx8.astype(np.float32)).astype(E4NP))

    in_maps = []
    for core in range(NCORES):
        g, r = core // 4, core % 4
        h0 = r * HL
        qcols = slice(h0 * D, (h0 + HL) * D)
        kcols = slice(C + h0 * D, C + (h0 + HL) * D)
        vcols = slice(2 * C + h0 * D, 2 * C + (h0 + HL) * D)
        wqk_s = np.concatenate(
            [w_attn[:, qcols], w_attn[:, kcols]], axis=1) * AW
        wqk8_np = _chunked(_q8(wqk_s).astype(np.float32)).astype(E4NP)
        wv_s = np.ascontiguousarray(w_attn[:, vcols]) * AW
        wv8 = _q8(wv_s)
        dwv8 = _q8(wv_s - wv8.astype(np.float32))
        wp_s = np.ascontiguousarray(w_proj[:, 256 * r: 256 * (r + 1)]) * AW
        wp8 = _q8(wp_s)
        dwp8 = _q8(wp_s - wp8.astype(np.float32))
        m = {
            "xt8": xt8_g[g],
            "dxt8": dxt8_g[g],
            "wqk8": wqk8_np,
            "wv8": _chunked(wv8.astype(np.float32)).astype(E4NP),
            "dwv8": _chunked(dwv8.astype(np.float32)).astype(E4NP),
            "wp8": _chunked(wp8.astype(np.float32)).astype(E4NP),
            "dwp8": _chunked(dwp8.astype(np.float32)).astype(E4NP),
            "tri": tri_np,
            "vone": vone_np,
            "sel": sel_np,
        }
        if qk_bias:
            bq = np.concatenate([b_attn[qcols], b_attn[kcols]]) * AS  # (512,)
            m["bqk"] = np.ascontiguousarray(
                bq.reshape(4, 128).T.astype(np.float32))
        if out_bias:
            bo = b_out_full[256 * r: 256 * (r + 1)]
            m["bout"] = np.ascontiguousarray(
                bo.reshape(2, 128).T.astype(np.float32))
        in_maps.append(m)
    return in_maps, qk_bias, out_bias


def assemble_output(results):
    out = np.empty((B, T, C), dtype=np.float32)
    for core in range(NCORES):
        g, r = core // 4, core % 4
        out[g][:, 256 * r: 256 * (r + 1)] = results[core]["out_c"].T
    return out


def kernel(x, w_attn, b_attn, w_proj, b_proj):
    from concourse.bass_utils import run_bass_kernel_spmd

    in_maps, qk_bias, out_bias = make_in_maps(
        x, w_attn, b_attn, w_proj, b_proj)
    nc = build_program(reps=1, qk_bias=qk_bias, out_bias=out_bias)
    res = run_bass_kernel_spmd(nc, in_maps, list(range(NCORES)))
    return assemble_output(res.results)


# revision 28
# speedup vs baseline: 1.1704x; 1.1126x over previous
"""Causal self-attention (B=2, T=2048, C=1024, H=16, D=64) on 8 TRN2 cores.

Sharding: batch across 2 groups of 4 cores; 4 heads per core within a group
(Megatron column-parallel QKV). After attention, AllGather the per-head
outputs within each group (fp8 value+residual pair), then column-parallel
c_proj (each core computes 256 output columns for all T).

fp8 strategy (all scale factors are exact powers of two, folded into
existing copies / the exp scale, so descale costs nothing):
  X' = x^T*AX quantized to fp8e4m3 host-side as x8 + residual dx8.
  W' = w*AW quantized host-side (wqk8 single; wv8+dwv8, wp8+dwp8 pairs).
  QK  = (x8 + dx8)@wqk8          2-pass DoubleRow (w-quant error is washed
                                 out by softmax; x-residual is not, so kept)
  V   = x8@wv8 + x8@dwv8 + dx8@wv8   3-pass DoubleRow (~exact: only the
                                 2nd-order dx8@dwv8 term is dropped)
  S   = q8^T k8 plain fp8 over D=64 (q,k requantized fp8*AS; softmax
                                 washes the quant error; DoubleRow scores
                                 measured slower on HW due to repack DMAs)
  att@V in fp32r (fp8 attention weights would cost ~1.4e-2 error alone).
  y -> y8 + dy8 fp8 pair (~exact); two packed [1024,512] AllGathers (chunk
  pairs) -- fewer group-sync points measured faster than four.
  proj = y8@wp8 + y8@dwp8 + dy8@wp8  3-pass DoubleRow (~exact)

Measured end-to-end rel err ~1.3e-2 vs the 2e-2 gate (numpy model 1.37e-2).

Per-core PE work drops from ~278K cycle-rows (all fp32r) to ~200K:
QKV 32.8K + V 24.6K + S 34.8K + att@V 69.6K(+diag) + norm 8.2K + proj 24.6K.

QKV (stage A) and attention (stage B) are emitted interleaved per t-chunk;
each chunk's normalize + AllGather is deferred by one chunk and issued
mid-compute so the first three AllGathers hide under attention. The
score/accumulator PSUM pools close right after attention so proj quarters
0-2 run before the final normalize+AllGather.

Output per core: out_c [256, 2048] = out^T columns slice; host reassembles.
"""

import sys

sys.path.insert(0, "/opt/trn_rl_repo")

from contextlib import ExitStack

import numpy as np
import ml_dtypes

B, T, C, H, D = 2, 2048, 1024, 16, 64
NCORES = 8
HL = 4   # heads per core
NKC = 8  # contraction chunks (C / 128)
NCH = 4  # t chunks (T / 512)
NST = 16  # s tiles (T / 128)

AX = 8.0    # x fp8 scale
AW = 256.0  # weight fp8 scale
AS = 16.0   # q/k score-input fp8 scale
AY = 32.0   # y fp8 scale

E4NP = ml_dtypes.float8_e4m3

_prog_cache = {}

# ablation flags for perf experiments (not used in production path)
VARIANT = {"no_ag": False, "no_proj": False, "scores_nodr": True,
           "ag_sync": False, "bufs4": False, "big_nodr": False,
           "ag2": True, "swpipe": True, "ag3": True}


def build_program(reps=1, qk_bias=False, out_bias=False):
    key = (reps, qk_bias, out_bias, tuple(sorted(VARIANT.items())))
    if key in _prog_cache:
        return _prog_cache[key]

    from concourse import bacc, mybir
    import concourse.tile as tile

    F32 = mybir.dt.float32
    F32R = mybir.dt.float32r
    F8 = mybir.dt.float8e4

    nc = bacc.Bacc(num_devices=NCORES)

    xt8 = nc.declare_dram_parameter("xt8", [128, NKC, T], F8, isOutput=False)
    dxt8 = nc.declare_dram_parameter("dxt8", [128, NKC, T], F8, isOutput=False)
    wqk8 = nc.declare_dram_parameter("wqk8", [128, NKC, 512], F8, isOutput=False)
    wv8 = nc.declare_dram_parameter("wv8", [128, NKC, 256], F8, isOutput=False)
    dwv8 = nc.declare_dram_parameter("dwv8", [128, NKC, 256], F8, isOutput=False)
    wp8 = nc.declare_dram_parameter("wp8", [128, NKC, 256], F8, isOutput=False)
    dwp8 = nc.declare_dram_parameter("dwp8", [128, NKC, 256], F8, isOutput=False)
    tri = nc.declare_dram_parameter("tri", [128, 128], F32R, isOutput=False)
    vone = nc.declare_dram_parameter("vone", [128, NST, 4, 1], F32R, isOutput=False)
    sel = nc.declare_dram_parameter("sel", [16, 16, 64], F32R, isOutput=False)
    if qk_bias:
        bqk = nc.declare_dram_parameter("bqk", [128, 4], F32, isOutput=False)
    if out_bias:
        bout = nc.declare_dram_parameter("bout", [128, 2], F32, isOutput=False)
    out_c = nc.declare_dram_parameter("out_c", [256, T], F32, isOutput=True)

    with tile.TileContext(nc) as tc:
        with ExitStack() as outer:
            const = outer.enter_context(tc.tile_pool(name="const", bufs=1))
            wqk8_sb = const.tile([128, NKC, 512], F8)
            wv8_sb = const.tile([128, NKC, 256], F8)
            dwv8_sb = const.tile([128, NKC, 256], F8)
            wp8_sb = const.tile([128, NKC, 256], F8)
            dwp8_sb = const.tile([128, NKC, 256], F8)
            tri_sb = const.tile([128, 128], F32R)
            sel_sb = const.tile([16, 16, 64], F32R)
            nc.scalar.dma_start(wqk8_sb[:], wqk8[:])
            nc.scalar.dma_start(wv8_sb[:], wv8[:])
            nc.scalar.dma_start(dwv8_sb[:], dwv8[:])
            nc.scalar.dma_start(wp8_sb[:], wp8[:])
            nc.scalar.dma_start(dwp8_sb[:], dwp8[:])
            nc.scalar.dma_start(tri_sb[:], tri[:])
            nc.scalar.dma_start(sel_sb[:], sel[:])
            bqk_sb = bout_sb = None
            if qk_bias:
                bqk_sb = const.tile([128, 4], F32)
                nc.scalar.dma_start(bqk_sb[:], bqk[:])
            if out_bias:
                bout_sb = const.tile([128, 2], F32)
                nc.scalar.dma_start(bout_sb[:], bout[:])

            for rep in range(reps):
                _emit_body(
                    nc, tc, mybir, rep,
                    xt8=xt8, dxt8=dxt8, vone=vone, out_c=out_c,
                    wqk8_sb=wqk8_sb, wv8_sb=wv8_sb, dwv8_sb=dwv8_sb,
                    wp8_sb=wp8_sb, dwp8_sb=dwp8_sb,
                    tri_sb=tri_sb, sel_sb=sel_sb,
                    bqk_sb=bqk_sb, bout_sb=bout_sb,
                )

    nc.finalize()
    _prog_cache[key] = nc
    return nc


def _emit_body(nc, tc, mybir, rep, *, xt8, dxt8, vone, out_c, wqk8_sb,
               wv8_sb, dwv8_sb, wp8_sb, dwp8_sb, tri_sb, sel_sb,
               bqk_sb, bout_sb):
    F32 = mybir.dt.float32
    F32R = mybir.dt.float32r
    F8 = mybir.dt.float8e4
    AF = mybir.ActivationFunctionType
    MUL = mybir.AluOpType.mult
    SUB = mybir.AluOpType.subtract
    ADD = mybir.AluOpType.add
    DR = mybir.MatmulPerfMode.DoubleRow
    R = f"r{rep}"

    QK_SCALE = AS / (AX * AW)     # PSUM qk -> fp8 tile scale
    V_SCALE = 1.0 / (AX * AW)     # PSUM v -> f32r v_sb
    EXP_SCALE = 0.125 / (AS * AS)  # scores PSUM -> exp
    OUT_SCALE = 1.0 / (AY * AW)   # proj PSUM -> out

    with ExitStack() as persist:
        stP = persist.enter_context(tc.tile_pool(name=f"stP{R}", bufs=1))
        dpool = persist.enter_context(
            tc.tile_pool(name=f"dram{R}", bufs=1, space="DRAM"))
        if VARIANT["scores_nodr"]:
            # plain fp8 scores (K=64): [128=(hh,d), m, t] like v1's qk_sb
            q2_sb = stP.tile([128, 2, T], F8, name=f"q2_sb{R}")
            k2_sb = stP.tile([128, 2, T], F8, name=f"k2_sb{R}")
        else:
            # packed fp8 q/k for DoubleRow scores: [32, head, slot, t],
            # contraction d = 2p + slot
            q2_sb = stP.tile([32, 4, 2, T], F8, name=f"q2_sb{R}")
            k2_sb = stP.tile([32, 4, 2, T], F8, name=f"k2_sb{R}")
        # V natural f32r, 65-stride per head (65th col = ones)
        v_sb = stP.tile([128, NST, 260], F32R, name=f"v_sb{R}")
        # y raw + denominator row (partition 64), per t-chunk; blocks = head
        yraw_q = [
            stP.tile([65, 4, 512], F32R, name=f"yraw{R}_{q}")
            for q in range(NCH)
        ]
        # fp8 y8 (rows 0-255) + dy8 (rows 256-511) packed per chunk
        if VARIANT["ag3"]:
            # merged chunks 0+1, separate chunks 2 and 3 (tail AG half-size)
            y_in_q = [
                dpool.tile([1024, 512], F8, name=f"y_in{R}_0"),
                dpool.tile([512, 512], F8, name=f"y_in{R}_1"),
                dpool.tile([512, 512], F8, name=f"y_in{R}_2"),
            ]
            y_full_q = [
                dpool.tile([4096, 512], F8, name=f"y_full{R}_0"),
                dpool.tile([2048, 512], F8, name=f"y_full{R}_1"),
                dpool.tile([2048, 512], F8, name=f"y_full{R}_2"),
            ]
        elif VARIANT["ag2"]:
            y_in_q = [
                dpool.tile([1024, 512], F8, name=f"y_in{R}_{q}")
                for q in range(2)
            ]
            y_full_q = [
                dpool.tile([4096, 512], F8, name=f"y_full{R}_{q}")
                for q in range(2)
            ]
        else:
            y_in_q = [
                dpool.tile([512, 512], F8, name=f"y_in{R}_{q}")
                for q in range(NCH)
            ]
            y_full_q = [
                dpool.tile([2048, 512], F8, name=f"y_full{R}_{q}")
                for q in range(NCH)
            ]

        with (
            tc.tile_pool(name=f"stAB{R}", bufs=1) as stAB,
            tc.tile_pool(name=f"psA{R}", bufs=1, space="PSUM") as psA,
        ):
            sy_ctx = ExitStack()
            psS = sy_ctx.enter_context(
                tc.tile_pool(name=f"psS{R}", bufs=1, space="PSUM"))
            psY = sy_ctx.enter_context(
                tc.tile_pool(name=f"psY{R}", bufs=1, space="PSUM"))
            vview = v_sb[:].rearrange("p t (h x) -> p t h x", h=4)
            nc.scalar.dma_start(vview[:, :, :, 64:65], vone[:])
            xt_t, dxt_t = [], []
            for n in range(NCH):
                xtile = stAB.tile([128, NKC, 512], F8, tag="xt", bufs=3,
                                  name=f"xt_t{R}_{n}")
                nc.sync.dma_start(xtile[:], xt8[:, :, n * 512:(n + 1) * 512])
                xt_t.append(xtile)
                dxtile = stAB.tile([128, NKC, 512], F8, tag="dxt", bufs=3,
                                   name=f"dxt_t{R}_{n}")
                nc.sync.dma_start(dxtile[:], dxt8[:, :, n * 512:(n + 1) * 512])
                dxt_t.append(dxtile)

            def emit_norm_ag(n, yraw, r4):
                tmp = stAB.tile([64, 4, 512], F32, tag="tmp", bufs=2,
                                name=f"tmp{R}_{n}")
                for h in range(4):
                    rb = psA.tile([64, 512], F32, tag="pA", bufs=2,
                                  name=f"rb{R}_{n}_{h}")
                    nc.tensor.matmul(
                        rb[:], sel_sb[0:4, h, :], r4[:],
                        start=True, stop=True,
                    )
                    nc.vector.tensor_tensor(
                        tmp[:, h, :], yraw[0:64, h, :], rb[:], MUL,
                    )
                y8t = stAB.tile([64, 4, 512], F8, tag="y8", bufs=2,
                                name=f"y8{R}_{n}")
                nc.vector.tensor_copy(y8t[:], tmp[:])
                dy8t = stAB.tile([64, 4, 512], F8, tag="dy8", bufs=2,
                                 name=f"dy8{R}_{n}")
                nc.vector.tensor_tensor(dy8t[:], tmp[:], y8t[:], SUB)
                if VARIANT["ag3"]:
                    tin = y_in_q[0 if n < 2 else n - 1]
                    b0 = 512 * (n % 2) if n < 2 else 0
                elif VARIANT["ag2"]:
                    tin = y_in_q[n // 2]
                    b0 = 512 * (n % 2)
                else:
                    tin = y_in_q[n]
                    b0 = 0
                nc.scalar.dma_start(
                    tin[b0:b0 + 256, :].rearrange("(h p) u -> p h u", p=64),
                    y8t[:])
                nc.scalar.dma_start(
                    tin[b0 + 256:b0 + 512, :].rearrange(
                        "(h p) u -> p h u", p=64),
                    dy8t[:])
                if VARIANT["ag3"]:
                    fire = n != 0
                    oidx = 0 if n < 2 else n - 1
                elif VARIANT["ag2"]:
                    fire = n % 2 == 1
                    oidx = n // 2
                else:
                    fire = True
                    oidx = n
                if not VARIANT["no_ag"] and fire:
                    nc.gpsimd.collective_compute(
                        "AllGather",
                        mybir.AluOpType.bypass,
                        replica_groups=[[0, 1, 2, 3], [4, 5, 6, 7]],
                        ins=[tin[:]],
                        outs=[y_full_q[oidx][:]],
                    )

            pending = None
            for n in range(NCH):
                    yraw = yraw_q[n]
                    # --- QKV q/k part: 2-pass fp8 DoubleRow ---
                    for m in range(4):
                        ps = psA.tile([128, 512], F32, tag="pA", bufs=2,
                                      name=f"qkvps{R}_{n}_{m}")
                        for xt_pass, first, last in (
                            (xt_t[n], True, False),
                            (dxt_t[n], False, True),
                        ):
                            if VARIANT["big_nodr"]:
                                for kc in range(8):
                                    nc.tensor.matmul(
                                        ps[:],
                                        wqk8_sb[:, kc,
                                                m * 128:(m + 1) * 128],
                                        xt_pass[:, kc, :],
                                        start=(first and kc == 0),
                                        stop=(last and kc == 7),
                                    )
                            else:
                                for kcp in range(4):
                                    nc.tensor.matmul(
                                        ps[:],
                                        wqk8_sb[:, 2 * kcp:2 * kcp + 2,
                                                m * 128:(m + 1) * 128],
                                        xt_pass[:, 2 * kcp:2 * kcp + 2, :],
                                        start=(first and kcp == 0),
                                        stop=(last and kcp == 3),
                                        perf_mode=DR,
                                    )
                        if VARIANT["scores_nodr"]:
                            dst = q2_sb if m < 2 else k2_sb
                            tgt = dst[:, m % 2, n * 512:(n + 1) * 512]
                            if bqk_sb is not None:
                                nc.vector.tensor_scalar(
                                    tgt, ps[:], QK_SCALE,
                                    bqk_sb[:, m:m + 1], MUL, ADD,
                                )
                            else:
                                nc.vector.tensor_scalar_mul(
                                    tgt, ps[:], QK_SCALE)
                            continue
                        qk8t = stAB.tile(
                            [128, 512], F8, tag="qk8",
                            bufs=4 if VARIANT["bufs4"] else 3,
                            name=f"qk8{R}_{n}_{m}")
                        if bqk_sb is not None:
                            nc.vector.tensor_scalar(
                                qk8t[:], ps[:], QK_SCALE,
                                bqk_sb[:, m:m + 1], MUL, ADD,
                            )
                        else:
                            nc.vector.tensor_scalar_mul(
                                qk8t[:], ps[:], QK_SCALE)
                        # repack head halves -> [32, h, slot, t] with the
                        # d = 2p + slot packing (flat DMA orders match, so
                        # one dma_start per head)
                        dst = q2_sb if m < 2 else k2_sb
                        for hh in range(2):
                            h = 2 * (m % 2) + hh
                            nc.sync.dma_start(
                                dst[:, h, :, n * 512:(n + 1) * 512],
                                qk8t[64 * hh:64 * hh + 64, :],
                            )
                    # --- V: 3-pass fp8 DoubleRow ---
                    for tt in range(4):
                        st = 4 * n + tt
                        psv = psA.tile([128, 512], F32, tag="pA", bufs=2,
                                       name=f"vps{R}_{st}")
                        passes = (
                            (xt_t[n], wv8_sb, True, False),
                            (xt_t[n], dwv8_sb, False, False),
                            (dxt_t[n], wv8_sb, False, True),
                        )
                        for xa, wa, first, last in passes:
                            if VARIANT["big_nodr"]:
                                for kc in range(8):
                                    nc.tensor.matmul(
                                        psv[:, 0:256],
                                        xa[:, kc, tt * 128:(tt + 1) * 128],
                                        wa[:, kc, :],
                                        start=(first and kc == 0),
                                        stop=(last and kc == 7),
                                    )
                            else:
                                for kcp in range(4):
                                    nc.tensor.matmul(
                                        psv[:, 0:256],
                                        xa[:, 2 * kcp:2 * kcp + 2,
                                           tt * 128:(tt + 1) * 128],
                                        wa[:, 2 * kcp:2 * kcp + 2, :],
                                        start=(first and kcp == 0),
                                        stop=(last and kcp == 3),
                                        perf_mode=DR,
                                    )
                        nc.vector.tensor_scalar_mul(
                            v_sb[:, st, :].rearrange(
                                "p (h x) -> p h x", h=4)[:, :, 0:64],
                            psv[:, 0:256].rearrange("p (h x) -> p h x", h=4),
                            V_SCALE,
                        )

                    if pending is not None:
                        emit_norm_ag(*pending)
                        pending = None

                    n_st = 4 * n + 4
                    for p in range(2):
                        ype = psY.tile([65, 512], F32, tag="ye", bufs=1,
                                       name=f"ype{R}_{n}_{p}")
                        ypo = psY.tile([65, 512], F32, tag="yo", bufs=1,
                                       name=f"ypo{R}_{n}_{p}")
                        def emit_sc(st):
                            diag = st - 4 * n
                            toff = 128 * diag if diag >= 0 else 0
                            scp = psS.tile([128, 1024], F32, tag="sc", bufs=2,
                                           name=f"scp{R}_{n}_{p}_{st}")
                            es = stAB.tile(
                                [128, 1024], F32R, tag="es",
                                bufs=4 if VARIANT["bufs4"] else 3,
                                name=f"es{R}_{n}_{p}_{st}")
                            for hp in range(2):
                                h = 2 * p + hp
                                if VARIANT["scores_nodr"]:
                                    nc.tensor.matmul(
                                        scp[:, hp * 512 + toff:
                                            (hp + 1) * 512],
                                        k2_sb[64 * hp:64 * hp + 64, p,
                                              st * 128:(st + 1) * 128],
                                        q2_sb[64 * hp:64 * hp + 64, p,
                                              n * 512 + toff:(n + 1) * 512],
                                        start=True, stop=True,
                                    )
                                else:
                                    nc.tensor.matmul(
                                        scp[:, hp * 512 + toff:
                                            (hp + 1) * 512],
                                        k2_sb[:, h, :,
                                              st * 128:(st + 1) * 128],
                                        q2_sb[:, h, :,
                                              n * 512 + toff:(n + 1) * 512],
                                        start=True, stop=True,
                                        perf_mode=DR,
                                    )
                            if diag < 0:
                                nc.scalar.activation(
                                    es[:], scp[:], AF.Exp, scale=EXP_SCALE
                                )
                            else:
                                toff_ = toff
                                esv = es[:].rearrange(
                                    "p (hp u) -> p hp u", hp=2)
                                scv = scp[:].rearrange(
                                    "p (hp u) -> p hp u", hp=2)
                                nc.scalar.activation(
                                    esv[:, :, toff_:512],
                                    scv[:, :, toff_:512],
                                    AF.Exp, scale=EXP_SCALE,
                                )
                                for hp in range(2):
                                    nc.vector.tensor_tensor(
                                        es[:, hp * 512 + toff_:
                                           hp * 512 + toff_ + 128],
                                        es[:, hp * 512 + toff_:
                                           hp * 512 + toff_ + 128],
                                        tri_sb[:], MUL,
                                    )
                            return (st, toff, es)

                        def emit_av(item):
                            st, toff, es = item
                            for hp, yp in ((0, ype), (1, ypo)):
                                h = 2 * p + hp
                                nc.tensor.matmul(
                                    yp[:, toff:512],
                                    v_sb[:, st, 65 * h:65 * h + 65],
                                    es[:, hp * 512 + toff:(hp + 1) * 512],
                                    start=(st == 0), stop=(st == n_st - 1),
                                )

                        if VARIANT["swpipe"]:
                            # issue scores(st+1) before att@V(st) so the PE
                            # queue head never waits on exp(st) with ready
                            # score work behind it
                            prev = None
                            for st in range(n_st):
                                cur = emit_sc(st)
                                if prev is not None:
                                    emit_av(prev)
                                prev = cur
                            emit_av(prev)
                        else:
                            for st in range(n_st):
                                emit_av(emit_sc(st))
                        for hp, yp in ((0, ype), (1, ypo)):
                            h = 2 * p + hp
                            nc.vector.tensor_copy(yraw[:, h, :], yp[:])

                    den4 = stAB.tile([4, 512], F32R, tag="den4", bufs=2,
                                     name=f"den4{R}_{n}")
                    nc.scalar.dma_start(den4[:], yraw[64:65, :, :])
                    rf = stAB.tile([4, 512], F32, tag="rf", bufs=2,
                                   name=f"rf{R}_{n}")
                    nc.vector.reciprocal_approx_fast(
                        rf[:], den4[:].bitcast(F32))
                    r4 = stAB.tile([4, 512], F32R, tag="r4", bufs=2,
                                   name=f"r4{R}_{n}")
                    nc.vector.tensor_scalar_mul(r4[:], rf[:], AY)
                    pending = (n, yraw, r4)

            # free the attention score/accumulator banks (6), keep psA
            # open for the final normalize; proj 0-2 draws only from the
            # freed space, so it is NOT gated on the last recip chain.
            sy_ctx.close()

            def emit_proj(q, psP):
                pp0 = psP.tile([128, 512], F32, tag="pp0", bufs=2,
                               name=f"pp0{R}_{q}")
                pp1 = psP.tile([128, 512], F32, tag="pp1", bufs=2,
                               name=f"pp1{R}_{q}")
                for kcp in range(4):
                    # [p, slot j, w (y8 vs dy8), t]; one DMA per rank block
                    ydf = stAB.tile([128, 2, 2, 512], F8, tag="yf", bufs=4,
                                    name=f"ydf{R}_{q}_{kcp}")
                    if q < 3:
                        dma_eng = nc.sync
                    else:
                        dma_eng = nc.sync if kcp % 2 == 0 else nc.scalar
                    for w in range(2):
                        if VARIANT["ag3"]:
                            if q < 2:
                                ysrc = y_full_q[0]
                                base = 1024 * kcp + 512 * q + 256 * w
                            else:
                                ysrc = y_full_q[q - 1]
                                base = 512 * kcp + 256 * w
                        elif VARIANT["ag2"]:
                            ysrc = y_full_q[q // 2]
                            base = 1024 * kcp + 512 * (q % 2) + 256 * w
                        else:
                            ysrc = y_full_q[q]
                            base = 512 * kcp + 256 * w
                        dma_eng.dma_start(
                            ydf[:, :, w, :],
                            ysrc[base:base + 256, :].rearrange(
                                "(j p) u -> p j u", j=2),
                        )
                    yf = ydf[:, :, 0, :]
                    dyf = ydf[:, :, 1, :]
                    for m2, pp in ((0, pp0), (1, pp1)):
                        wsl = slice(m2 * 128, (m2 + 1) * 128)
                        if VARIANT["big_nodr"]:
                            for j in range(2):
                                kc = 2 * kcp + j
                                nc.tensor.matmul(
                                    pp[:], wp8_sb[:, kc, wsl],
                                    ydf[:, j, 0, :],
                                    start=(kcp == 0 and j == 0), stop=False,
                                )
                                nc.tensor.matmul(
                                    pp[:], dwp8_sb[:, kc, wsl],
                                    ydf[:, j, 0, :],
                                    start=False, stop=False,
                                )
                                nc.tensor.matmul(
                                    pp[:], wp8_sb[:, kc, wsl],
                                    ydf[:, j, 1, :],
                                    start=False,
                                    stop=(kcp == 3 and j == 1),
                                )
                        else:
                            ksl = slice(2 * kcp, 2 * kcp + 2)
                            nc.tensor.matmul(
                                pp[:], wp8_sb[:, ksl, wsl], yf,
                                start=(kcp == 0), stop=False, perf_mode=DR,
                            )
                            nc.tensor.matmul(
                                pp[:], dwp8_sb[:, ksl, wsl], yf,
                                start=False, stop=False, perf_mode=DR,
                            )
                            nc.tensor.matmul(
                                pp[:], wp8_sb[:, ksl, wsl], dyf,
                                start=False, stop=(kcp == 3), perf_mode=DR,
                            )
                out_sb = stAB.tile([128, 2, 512], F32, tag="out_sb", bufs=2,
                                   name=f"out_sb{R}_{q}")
                for m2, pp in ((0, pp0), (1, pp1)):
                    if bout_sb is not None:
                        nc.vector.tensor_scalar(
                            out_sb[:, m2, :], pp[:], OUT_SCALE,
                            bout_sb[:, m2:m2 + 1], MUL, ADD,
                        )
                    else:
                        nc.vector.tensor_scalar_mul(
                            out_sb[:, m2, :], pp[:], OUT_SCALE)
                nc.sync.dma_start(
                    out_c[:, q * 512:(q + 1) * 512].rearrange(
                        "(m p) t -> p m t", p=128),
                    out_sb[:],
                )

            with tc.tile_pool(name=f"psP{R}", bufs=1, space="PSUM") as psP:
                if VARIANT["no_proj"]:
                    emit_norm_ag(*pending)
                    junk = stAB.tile([128, 2, 512], F32, tag="out_sb", bufs=2,
                                     name=f"junk{R}")
                    nc.vector.memset(junk[:], 0.0)
                    for q in range(NCH):
                        nc.sync.dma_start(
                            out_c[:, q * 512:(q + 1) * 512].rearrange(
                                "(m p) t -> p m t", p=128),
                            junk[:],
                        )
                elif VARIANT["ag2"] or VARIANT["ag3"]:
                    # norm3+final AG first: their PE/DVE work would otherwise
                    # queue behind proj q0/q1, delaying the final gather
                    # whose transfer can hide under all four proj quarters
                    emit_norm_ag(*pending)
                    for q in range(4):
                        emit_proj(q, psP)
                else:
                    for q in range(3):
                        emit_proj(q, psP)
                    emit_norm_ag(*pending)
                    emit_proj(3, psP)


def _chunked(a):
    """(C, X) -> [128, C/128, X] contraction-chunked layout."""
    c, x = a.shape
    return np.ascontiguousarray(
        a.reshape(c // 128, 128, x).transpose(1, 0, 2)
    )


def _q8(a):
    return np.asarray(a, dtype=E4NP)


def make_in_maps(x, w_attn, b_attn, w_proj, b_proj):
    x = np.asarray(x, dtype=np.float32)
    w_attn = np.asarray(w_attn, dtype=np.float32)
    b_attn = np.asarray(b_attn, dtype=np.float32)
    w_proj = np.asarray(w_proj, dtype=np.float32)
    b_proj = np.asarray(b_proj, dtype=np.float32)

    qk_bias = bool(np.any(b_attn[: 2 * C] != 0))
    b_out_full = b_attn[2 * C:] @ w_proj + b_proj  # V bias folds through
    out_bias = bool(np.any(b_out_full != 0))

    tri_np = np.triu(np.ones((128, 128), np.float32))
    vone_np = np.ones((128, NST, 4, 1), np.float32)
    sel_np = np.zeros((16, 16, 64), np.float32)
    for i in range(16):
        sel_np[i, i, :] = 1.0

    xt8_g, dxt8_g = [], []
    for g in range(B):
        X = np.ascontiguousarray(x[g].T) * AX
        x8 = _q8(X)
        dx8 = _q8(X - x8.astype(np.float32))
        xt8_g.append(_chunked(x8.astype(np.float32)).astype(E4NP))
        dxt8_g.append(_chunked(dx8.astype(np.float32)).astype(E4NP))

    in_maps = []
    for core in range(NCORES):
        g, r = core // 4, core % 4
        h0 = r * HL
        qcols = slice(h0 * D, (h0 + HL) * D)
        kcols = slice(C + h0 * D, C + (h0 + HL) * D)
        vcols = slice(2 * C + h0 * D, 2 * C + (h0 + HL) * D)
        wqk_s = np.concatenate(
            [w_attn[:, qcols], w_attn[:, kcols]], axis=1) * AW
        wqk8_np = _chunked(_q8(wqk_s).astype(np.float32)).astype(E4NP)
        wv_s = np.ascontiguousarray(w_attn[:, vcols]) * AW
        wv8 = _q8(wv_s)
        dwv8 = _q8(wv_s - wv8.astype(np.float32))
        wp_s = np.ascontiguousarray(w_proj[:, 256 * r: 256 * (r + 1)]) * AW
        wp8 = _q8(wp_s)
        dwp8 = _q8(wp_s - wp8.astype(np.float32))
        m = {
            "xt8": xt8_g[g],
            "dxt8": dxt8_g[g],
            "wqk8": wqk8_np,
            "wv8": _chunked(wv8.astype(np.float32)).astype(E4NP),
            "dwv8": _chunked(dwv8.astype(np.float32)).astype(E4NP),
            "wp8": _chunked(wp8.astype(np.float32)).astype(E4NP),
            "dwp8": _chunked(dwp8.astype(np.float32)).astype(E4NP),
            "tri": tri_np,
            "vone": vone_np,
            "sel": sel_np,
        }
        if qk_bias:
            bq = np.concatenate([b_attn[qcols], b_attn[kcols]]) * AS  # (512,)
            m["bqk"] = np.ascontiguousarray(
                bq.reshape(4, 128).T.astype(np.float32))
        if out_bias:
            bo = b_out_full[256 * r: 256 * (r + 1)]
            m["bout"] = np.ascontiguousarray(
                bo.reshape(2, 128).T.astype(np.float32))
        in_maps.append(m)
    return in_maps, qk_bias, out_bias


def assemble_output(results):
    out = np.empty((B, T, C), dtype=np.float32)
    for core in range(NCORES):
        g, r = core // 4, core % 4
        out[g][:, 256 * r: 256 * (r + 1)] = results[core]["out_c"].T
    return out


def kernel(x, w_attn, b_attn, w_proj, b_proj):
    from concourse.bass_utils import run_bass_kernel_spmd

    in_maps, qk_bias, out_bias = make_in_maps(
        x, w_attn, b_attn, w_proj, b_proj)
    nc = build_program(reps=1, qk_bias=qk_bias, out_bias=out_bias)
    res = run_bass_kernel_spmd(nc, in_maps, list(range(NCORES)))
    return assemble_output(res.results)
